# revision 1
# baseline (speedup 1.0000x reference)
"""DeformConv (B=8, C=256, H=W=64, O=256, 3x3, DG=1) Trainium2 Bass kernel.

Sharding: data-parallel over batch, one batch element per NeuronCore (8 cores).

Per-core pipeline (B=1):
  1. x [256,4096] f32 -> fp16 (SWDGE cast-load) -> PE-transpose to
     xt_sb [4096pos, 256ch] fp16 -> build a 2x2-patch table in DRAM:
     x_patch[lin] = [x_t[lin], x_t[lin+1], x_t[lin+64], x_t[lin+65]] (2KB rows)
     via 7 shifted strided DMA writes.
  2. Coords on DVE (f32): y0=floor(sy) (magic-number round + is_gt fix),
     base row r=clip(y0,0,62), col b=clip(x0,0,62), separable slot weights
     wsy[2], wsx[2] reproducing mmcv zero-padding bilinear exactly.
  3. Gather: one dma_gather per (1024-pos chunk, tap): 2KB elems from
     x_patch, alternating 2 SWDGE queues. Output [pos%128, pos//128, 4*256].
  4. Blend: per corner, broadcast ws along channels (free-step-0 AP copy)
     then big [128,8,256] tensor_tensor mult/add on DVE.
  5. PE-transpose blended [pos,ch]->[ch,pos] (fp16) + ACT copies PSUM->SBUF.
  6. GEMM out[o,p] = sum_{c,k} W[o,c,k]*sampled[c,k,p]: 18 contraction
     blocks of 128, fp16 operands, f32 PSUM.
"""

import dataclasses

import numpy as np

_CACHE = {}

H = 64
W = 64
HW = 4096
C = 256
O = 256
K = 9
NCORES = 8
MAGIC = float(3 << 22)  # 1.5*2^23: keeps x+MAGIC in [2^23, 2^24) for |x|<2^22
USE_XT_XBAR = False  # x transpose via DMA xbar instead of PE
USE_BAND0 = True  # chunk-0 gathers from the early-built band-0 prefix table
USE_ILGEMM = True  # interleave GEMM accumulation into the tap loop


def _step0(ap, inner):
    """Expand a [128, n] AP to [128, n, inner] with stride-0 inner dim."""
    return dataclasses.replace(ap, ap=list(ap.ap) + [[0, inner]])


def _emit(tc, nc, aps, rec=None, queue_plan=None):
    import contextlib

    import concourse.bass as bass
    import concourse.mybir as mybir
    from concourse.masks import make_identity

    dt = mybir.dt
    Alu = mybir.AluOpType
    Act = mybir.ActivationFunctionType

    x_in = aps["x"]          # [256, 4096] f32
    off_in = aps["offset"]   # [18, 4096]  f32
    w2_in = aps["w2"]        # [2304, 256] f32   (k-major, then c; lhsT layout)
    out_d = aps["out"]       # [256, 4096] f32

    ctx = contextlib.ExitStack()
    with ctx:
        # ---------------- pools ----------------
        cpool = ctx.enter_context(tc.tile_pool(name="cpool", bufs=1))
        dpool = ctx.enter_context(tc.tile_pool(name="dpool", bufs=1, space="DRAM"))

        # ---------------- persistent tiles ----------------
        ident16 = cpool.tile([128, 128], dt.float16, name="ident16")
        ident32 = cpool.tile([128, 128], dt.float32, name="ident32")
        make_identity(nc, ident16)
        make_identity(nc, ident32)

        w2_sb = cpool.tile([128, 18, 256], dt.float16, name="w2_sb")
        _i = nc.gpsimd.dma_start(
            out=w2_sb, in_=w2_in.rearrange("(kb ci) o -> ci kb o", ci=128)
        )
        if rec is not None:
            rec["plain"].append(_i.ins if hasattr(_i, "ins") else _i)
        # slot-weight fields: corners 1-3 stored as adjacent PAIRS so the
        # blend mults' in1 AP has innermost step-1 (enables DVE 2x_1P mode);
        # corner 0 stays f32 per-partition scalars for the ACT path.
        ws16p = cpool.tile([128, 3, K * 32, 2], dt.float16, name="ws16p")
        ws32 = cpool.tile([128, 1, K * 32], dt.float32, name="ws32")
        idxw = cpool.tile([128, K * 4 * 64], dt.int16, name="idxw")
        gsem = [nc.alloc_semaphore(f"gsem{q}") for q in range(2)]

        x_patch = dpool.tile([HW, 1024], dt.float16, name="x_patch")
        x_band0 = dpool.tile([1536, 1024], dt.float16, name="x_band0")


        # ================= PREP PHASE (scoped pools) =================
        with tc.tile_pool(name="prep", bufs=1) as pp, tc.tile_pool(
            name="ppsum", bufs=2, space="PSUM"
        ) as pps:
            # ---- offsets -> p-major layout via PE transpose ----
            off_sb = pp.tile([18, HW], dt.float32, name="off_sb")
            nc.sync.dma_start(out=off_sb, in_=off_in)
            offp = pp.tile([128, 32, 18], dt.float32, name="offp")
            for i in range(32):
                pso = pps.tile([128, 18], dt.float32, name="pso", tag="pso")
                nc.tensor.transpose(
                    pso, off_sb[:, i * 128 : (i + 1) * 128], ident32[0:18, 0:18]
                )
                nc.vector.tensor_copy(offp[:, i, :], pso)

            # ---- position iota ----
            pos_i = pp.tile([128, 32], dt.int32, name="pos_i")
            nc.gpsimd.iota(pos_i, pattern=[[128, 32]], base=0, channel_multiplier=1)
            POS = pp.tile([128, 32], dt.float32, name="POS")
            nc.vector.tensor_copy(POS, pos_i)
            Pq = pp.tile([128, 32], dt.float32, name="Pq")
            nc.vector.tensor_scalar(Pq, POS, 1.0 / 64.0, None, Alu.mult)
            I_ = pp.tile([128, 32], dt.float32, name="I_")
            CMP = pp.tile([128, 32], dt.float32, name="CMPij")
            nc.vector.tensor_scalar(CMP, Pq, MAGIC, None, Alu.add)
            nc.vector.tensor_scalar(I_, CMP, MAGIC, None, Alu.subtract)
            nc.vector.tensor_tensor(CMP, I_, Pq, Alu.is_gt)
            nc.vector.tensor_tensor(I_, I_, CMP, Alu.subtract)
            J_ = pp.tile([128, 32], dt.float32, name="J_")
            nc.vector.scalar_tensor_tensor(J_, I_, -64.0, POS, Alu.mult, Alu.add)

            # ---- x load + cast, then DMA-xbar transpose -> xt_sb [pos, ch] ----
            x_sb = pp.tile([128, 2, HW], dt.float16, name="x_sb")
            _i = nc.gpsimd.dma_start(
                out=x_sb, in_=x_in.rearrange("(h c) p -> c h p", h=2)
            )
            if rec is not None:
                rec["plain"].append(_i.ins if hasattr(_i, "ins") else _i)
            eng = [nc.sync, nc.scalar]
            xt_sb = pp.tile([128, 32, C], dt.float16, name="xt_sb")
            if USE_XT_XBAR:
                for h in range(2):
                    xt_dst = bass.AP(
                        tensor=xt_sb.tensor,
                        offset=xt_sb.offset + h * 128,
                        ap=[xt_sb.ap[0], [256, 32], [1, 128]],
                    )
                    eng[h].dma_start_transpose(out=xt_dst, in_=x_sb[:, h, :])
            else:
                for i in range(32):
                    xtp = pps.tile([128, 256], dt.float16, name="xtp", tag="xtp")
                    for h in range(2):
                        nc.tensor.transpose(
                            xtp[:, h * 128 : (h + 1) * 128],
                            x_sb[:, h, i * 128 : (i + 1) * 128],
                            ident16,
                        )
                    nc.scalar.activation(xt_sb[:, i, :], xtp, Act.Copy)

            # ---- per-axis coordinate pipeline ----
            KI = [k // 3 for k in range(K)]
            KJ = [k % 3 for k in range(K)]

            def axis_pipeline(off_field, base_tile, kshift, L, WS0, WS1, R_out):
                F = K * 32
                S = pp.tile([128, F], dt.float32, name=f"S{L}", tag=f"S{L}")
                for k in range(K):
                    nc.vector.scalar_tensor_tensor(
                        S[:, k * 32 : (k + 1) * 32],
                        off_field(k),
                        float(kshift[k] - 1),
                        base_tile,
                        Alu.add,
                        Alu.add,
                    )
                t = lambda nm: pp.tile([128, F], dt.float32, name=nm, tag=nm)
                Y0 = t(f"Y0{L}")
                Ct = t(f"Ct{L}")
                nc.vector.tensor_scalar(Ct, S, MAGIC, None, Alu.add)
                nc.vector.tensor_scalar(Y0, Ct, MAGIC, None, Alu.subtract)
                nc.vector.tensor_tensor(Ct, Y0, S, Alu.is_gt)
                nc.vector.tensor_tensor(Y0, Y0, Ct, Alu.subtract)
                LY = t(f"LY{L}")
                nc.vector.tensor_tensor(LY, S, Y0, Alu.subtract)
                WY0 = t(f"WY0{L}")
                nc.vector.tensor_scalar(WY0, LY, -1.0, 1.0, Alu.mult, Alu.add)
                V0 = t(f"V0{L}")
                V1 = t(f"V1{L}")
                nc.vector.tensor_scalar(V0, Y0, 0.0, None, Alu.is_ge)
                nc.vector.tensor_scalar(Ct, Y0, 63.0, None, Alu.is_le)
                nc.vector.tensor_tensor(V0, V0, Ct, Alu.mult)
                nc.vector.tensor_scalar(V1, Y0, -1.0, None, Alu.is_ge)
                nc.vector.tensor_scalar(Ct, Y0, 62.0, None, Alu.is_le)
                nc.vector.tensor_tensor(V1, V1, Ct, Alu.mult)
                nc.vector.tensor_tensor(WY0, WY0, V0, Alu.mult)
                nc.vector.tensor_tensor(LY, LY, V1, Alu.mult)
                R = R_out
                nc.vector.tensor_scalar(R, Y0, 0.0, 62.0, Alu.max, Alu.min)
                C0 = t(f"C0{L}")
                C1 = t(f"C1{L}")
                nc.vector.tensor_scalar(C0, Y0, 0.0, 63.0, Alu.max, Alu.min)
                nc.vector.tensor_scalar(C1, Y0, 1.0, 0.0, Alu.add, Alu.max)
                nc.vector.tensor_scalar(C1, C1, 63.0, None, Alu.min)
                E = t(f"E{L}")
                T1 = t(f"T1{L}")
                nc.vector.tensor_tensor(E, C0, R, Alu.is_equal)
                nc.vector.tensor_tensor(T1, WY0, E, Alu.mult)
                nc.vector.tensor_tensor(E, C1, R, Alu.is_equal)
                nc.vector.tensor_tensor(E, LY, E, Alu.mult)
                nc.vector.tensor_tensor(WS0, T1, E, Alu.add)
                Rp = t(f"Rp{L}")
                nc.vector.tensor_scalar(Rp, R, 1.0, None, Alu.add)
                nc.vector.tensor_tensor(E, C0, Rp, Alu.is_equal)
                nc.vector.tensor_tensor(T1, WY0, E, Alu.mult)
                nc.vector.tensor_tensor(E, C1, Rp, Alu.is_equal)
                nc.vector.tensor_tensor(E, LY, E, Alu.mult)
                nc.vector.tensor_tensor(WS1, T1, E, Alu.add)

            F = K * 32
            WSY0 = pp.tile([128, F], dt.float32, name="WSY0")
            WSY1 = pp.tile([128, F], dt.float32, name="WSY1")
            WSX0 = pp.tile([128, F], dt.float32, name="WSX0")
            WSX1 = pp.tile([128, F], dt.float32, name="WSX1")
            RY = pp.tile([128, F], dt.float32, name="RY")
            RX = pp.tile([128, F], dt.float32, name="RX")
            axis_pipeline(lambda k: offp[:, :, 2 * k], I_, KI, "y", WSY0, WSY1, RY)
            axis_pipeline(
                lambda k: offp[:, :, 2 * k + 1], J_, KJ, "x", WSX0, WSX1, RX
            )
            WSf = pp.tile([128, F], dt.float32, name="WSf", tag="WSf")
            nc.vector.tensor_tensor(ws32[:, 0, :], WSY0, WSX0, Alu.mult)
            for st, (wy, wx) in enumerate(
                [(None, None), (WSY0, WSX1), (WSY1, WSX0), (WSY1, WSX1)]
            ):
                if st == 0:
                    continue
                nc.vector.tensor_tensor(WSf, wy, wx, Alu.mult)
                nc.vector.tensor_copy(ws16p[:, st - 1, :, 0], WSf)
                nc.vector.tensor_copy(ws16p[:, st - 1, :, 1], WSf)

            # ---- gather indices: lin = RY*64 + RX, cast to i16 ----
            IDX = pp.tile([128, 384], dt.float32, name="IDX")
            nc.gpsimd.memset(IDX, 0)
            nc.vector.scalar_tensor_tensor(
                IDX[:, 0:F], RY, 64.0, RX, Alu.mult, Alu.add
            )
            # shuffle p%128 -> p%16 wrap via two PE transpose stages (f32),
            # casting to i16 on the final PSUM->SBUF copy:
            # idxw[t, (k,ch)*64 + bl*8 + g] = IDX[g*16+t, k*32+ch*8+bl]
            t1sb = pp.tile([128, 3, 128], dt.float32, name="t1sb")
            for ct in range(3):
                ps1 = pps.tile([128, 128], dt.float32, name="ps1", tag="ps1")
                nc.tensor.transpose(ps1, IDX[:, ct * 128 : (ct + 1) * 128], ident32)
                nc.vector.tensor_copy(t1sb[:, ct, :], ps1)
            # stage 2: per (ct, g): [128col, 16] -> [16, 128col]
            for ct in range(3):
                nk = 4 if ct < 2 else 1  # k-count covered by this col tile
                for g in range(8):
                    ps2 = pps.tile([16, 128], dt.float32, name="ps2", tag="ps2")
                    nc.tensor.transpose(
                        ps2, t1sb[:, ct, g * 16 : (g + 1) * 16], ident32
                    )
                    # dst cols: for k' in [0,nk), ch in 4, bl in 8:
                    #   ((ct*4+k')*4+ch)*64 + bl*8 + g
                    dst = bass.AP(
                        tensor=idxw.tensor,
                        offset=idxw.offset + (ct * 4 * 4) * 64 + g,
                        ap=[[idxw.ap[0][0], 16], [256, nk], [64, 4], [8, 8]],
                    )
                    nc.vector.tensor_copy(
                        dst,
                        ps2[0:16, 0 : nk * 32].rearrange(
                            "t (k c b) -> t k c b", k=nk, c=4
                        ),
                    )
            # replicate the wrapped idx table to all 8 partition groups on
            # the gpsimd SWDGE queue: keeps the sync/scalar HWDGE queues
            # free for the patch-table builds (and vice versa).
            for rep in range(1, 8):
                _i = nc.gpsimd.dma_start(
                    out=idxw[rep * 16 : (rep + 1) * 16, :], in_=idxw[0:16, :]
                )
                if rec is not None:
                    rec["plain"].append(_i.ins if hasattr(_i, "ins") else _i)

            # ---- patch tables (on sync/scalar, not gated behind idxw).
            # x_band0 = rows [0,1536) only: unblocks chunk-0 gathers early;
            # the full x_patch build overlaps with chunk 0-1 compute.
            # x_patch[lin, (s,t)*256:+256] = xt[lin+64s+t]
            if USE_BAND0:
                for s in range(2):
                    for t in range(2):
                        sh = 64 * s + t
                        slot = (2 * s + t) * 256
                        dst_a = bass.AP(
                            tensor=x_band0.tensor,
                            offset=x_band0.offset + slot,
                            ap=[[1024, 128 - sh], [128 * 1024, 12], [1, 256]],
                        )
                        eng[(2 * s + t) % 2].dma_start(
                            out=dst_a, in_=xt_sb[sh:128, 0:12, :]
                        )
                        if sh:
                            dst_b = bass.AP(
                                tensor=x_band0.tensor,
                                offset=x_band0.offset + slot + (128 - sh) * 1024,
                                ap=[[1024, sh], [128 * 1024, 12], [1, 256]],
                            )
                            eng[(2 * s + t + 1) % 2].dma_start(
                                out=dst_b, in_=xt_sb[0:sh, 1:13, :]
                            )
            for s in range(2):
                for t in range(2):
                    sh = 64 * s + t
                    slot = (2 * s + t) * 256
                    # rows p = i*128+j ; dst row p-sh for p >= sh
                    for half in range(2):
                        i0 = half * 16
                        dst_a = bass.AP(
                            tensor=x_patch.tensor,
                            offset=x_patch.offset + slot + i0 * 128 * 1024,
                            ap=[[1024, 128 - sh], [128 * 1024, 16], [1, 256]],
                        )
                        eng[(2 * s + t + half) % 2].dma_start(
                            out=dst_a, in_=xt_sb[sh:128, i0 : i0 + 16, :]
                        )
                    if sh:
                        dst_b = bass.AP(
                            tensor=x_patch.tensor,
                            offset=x_patch.offset + slot + (128 - sh) * 1024,
                            ap=[[1024, sh], [128 * 1024, 31], [1, 256]],
                        )
                        eng[(s + t) % 2].dma_start(
                            out=dst_b, in_=xt_sb[0:sh, 1:32, :]
                        )

        # ================= MAIN LOOP =================
        pspool = ctx.enter_context(tc.tile_pool(name="pspool", bufs=3, space="PSUM"))
        psg = ctx.enter_context(tc.tile_pool(name="psg", bufs=1, space="PSUM"))
        gpool = ctx.enter_context(tc.tile_pool(name="gpool", bufs=2))
        spool = ctx.enter_context(tc.tile_pool(name="spool", bufs=2))
        bpool = ctx.enter_context(tc.tile_pool(name="bpool", bufs=3))
        opool = ctx.enter_context(tc.tile_pool(name="opool", bufs=3))

        for ch in range(4):  # 1024-position chunks
            S = [
                spool.tile([128, 1024], dt.float16, name=f"S{kb}", tag=f"S{kb}")
                for kb in range(18)
            ]
            pgs = [
                psg.tile([128, 512], dt.float32, name=f"pg{j}", tag=f"pg{j}")
                for j in range(4)
            ]
            for k in range(K):
                G = gpool.tile([128, 8, 1024], dt.float16, name="G", tag="G", bufs=3)
                qi = ch * K + k
                qn = 0 if queue_plan is None else queue_plan[qi]
                _i = nc.gpsimd.dma_gather(
                    G,
                    x_band0 if (USE_BAND0 and ch == 0) else x_patch,
                    idxw[:, (k * 4 + ch) * 64 : (k * 4 + ch + 1) * 64],
                    num_idxs=1024,
                    num_idxs_reg=1024,
                    elem_size=1024,
                    elem_step=1024,
                    queue_num=qn,
                )
                if rec is not None:
                    rec["gather"].append(_i.ins if hasattr(_i, "ins") else _i)
                # blend 4 corners: A = sum_st ws_st * G[:, :, st].
                # corner 0 products on ACT (per-partition scale, per-bl ops);
                # corners 1-3 on DVE as fused broadcast-mults (step-0 in1).
                A = bpool.tile([128, 8, 256], dt.float16, name="A", tag="A")
                Mt = bpool.tile([128, 8, 256], dt.float16, name="Mt", tag="Mt")
                P0 = bpool.tile([128, 8, 256], dt.float16, name="P0", tag="P0")
                for bl in range(8):
                    wc = k * 32 + ch * 8 + bl
                    nc.scalar.activation(
                        P0[:, bl, :],
                        G[:, bl, 0:256],
                        Act.Copy,
                        scale=ws32[:, 0, wc : wc + 1],
                    )
                for st in range(1, 4):
                    # in1: [part, bl(x2 step), 128(x step0), pair(step1)] —
                    # innermost step-1 fp16 pairs keep DVE in 2x_1P mode.
                    wsl = ws16p[:, st - 1, k * 32 + ch * 8 : k * 32 + ch * 8 + 8, :]
                    wpair = dataclasses.replace(
                        wsl, ap=[wsl.ap[0], [2, 8], [0, 128], [1, 2]]
                    )
                    dst = Mt if st > 1 else A
                    nc.vector.tensor_tensor(
                        dst,
                        G[:, :, st * 256 : (st + 1) * 256],
                        wpair,
                        Alu.mult,
                    )
                    if st == 1:
                        nc.vector.tensor_tensor(A, A, P0, Alu.add)
                    else:
                        nc.vector.tensor_tensor(A, A, Mt, Alu.add)
                # transpose [pos, ch] -> [ch, pos]
                for h in range(2):
                    for blq in range(2):
                        pt = pspool.tile(
                            [128, 512], dt.float16, name="pt", tag="pt", bufs=3
                        )
                        for bb in range(4):
                            bl = blq * 4 + bb
                            nc.tensor.transpose(
                                pt[:, bb * 128 : (bb + 1) * 128],
                                A[:, bl, h * 128 : (h + 1) * 128],
                                ident16,
                            )
                        nc.scalar.activation(
                            S[k * 2 + h][:, blq * 512 : (blq + 1) * 512],
                            pt,
                            Act.Copy,
                        )
                # interleaved GEMM, 1-tap delayed: tap k's 2 kb-blocks are
                # contracted while tap k+1's transposes provide PE slack,
                # so matmuls never wait on the same tap's PSUM->SBUF copies.
                # 8 [128x512] matmuls per tap keeps PE load even.
                if USE_ILGEMM:
                    gk = [k - 1] if k >= 1 else []
                    if k == K - 1:
                        gk.append(k)
                    for kk in gk:
                        for m in range(2):
                            for sub in range(2):
                                for kb in (kk * 2, kk * 2 + 1):
                                    nc.tensor.matmul(
                                        pgs[m * 2 + sub],
                                        lhsT=w2_sb[:, kb, m * 128 : (m + 1) * 128],
                                        rhs=S[kb][:, sub * 512 : (sub + 1) * 512],
                                        start=(kb == 0),
                                        stop=(kb == 17),
                                    )
            if not USE_ILGEMM:
                for m in range(2):
                    for sub in range(2):
                        for kb in range(18):
                            nc.tensor.matmul(
                                pgs[m * 2 + sub],
                                lhsT=w2_sb[:, kb, m * 128 : (m + 1) * 128],
                                rhs=S[kb][:, sub * 512 : (sub + 1) * 512],
                                start=(kb == 0),
                                stop=(kb == 17),
                            )
            for m in range(2):
                for sub in range(2):
                    ot = opool.tile([128, 512], dt.float32, name="ot", tag="ot")
                    nc.vector.tensor_copy(ot, pgs[m * 2 + sub])
                    nc.sync.dma_start(
                        out=out_d[
                            m * 128 : (m + 1) * 128,
                            ch * 1024 + sub * 512 : ch * 1024 + (sub + 1) * 512,
                        ],
                        in_=ot,
                    )


def _lane_of(inst):
    from concourse.tile_sem_assignment import PROC_NAME_TO_IDX

    rev = {v: k for k, v in PROC_NAME_TO_IDX.items()}
    nm = rev.get(inst.bass_scheduled_proc, "")
    return int(nm[5:]) if nm.startswith("DMASW") else None


def build(queue_plan="auto"):
    import concourse.mybir as mybir
    from concourse import bacc, tile

    dt = mybir.dt
    nc = bacc.Bacc(
        "TRN2",
        target_bir_lowering=False,
        debug=False,
        enable_asserts=False,
        num_devices=NCORES,
        num_swdge_queues=2,
    )
    aps = {
        "x": nc.dram_tensor("x", [C, HW], dt.float32, kind="ExternalInput").ap(),
        "offset": nc.dram_tensor(
            "offset", [2 * K, HW], dt.float32, kind="ExternalInput"
        ).ap(),
        "w2": nc.dram_tensor(
            "w2", [C * K, O], dt.float32, kind="ExternalInput"
        ).ap(),
        "out": nc.dram_tensor(
            "out", [O, HW], dt.float32, kind="ExternalOutput"
        ).ap(),
    }
    if queue_plan == "auto":
        # pass 1: discover each SWDGE DMA's DMASW lane, then rebuild with a
        # lane-consistent queue assignment (lane%2, forced 0 on lanes that
        # host plain queue-0 dma_starts).
        rec = {"gather": [], "plain": []}
        with tile.TileContext(nc) as tc:
            _emit(tc, nc, aps, rec=rec, queue_plan=None)
        plain_lanes = {_lane_of(i) for i in rec["plain"]}
        plan = []
        for gi in rec["gather"]:
            lane = _lane_of(gi)
            q = 0 if (lane is None or lane in plain_lanes) else lane % 2
            plan.append(q)
        return build(plan)
    with tile.TileContext(nc) as tc:
        _emit(tc, nc, aps, queue_plan=queue_plan)
    nc.compile()
    return nc


def prep_in_maps(x, offset, weight):
    x = np.asarray(x, dtype=np.float32)
    offset = np.asarray(offset, dtype=np.float32)
    weight = np.asarray(weight, dtype=np.float32)
    w2 = np.ascontiguousarray(
        weight.reshape(O, C, K).transpose(2, 1, 0).reshape(C * K, O)
    )
    in_maps = []
    for b in range(NCORES):
        in_maps.append(
            {
                "x": np.ascontiguousarray(x[b].reshape(C, HW)),
                "offset": np.ascontiguousarray(offset[b].reshape(2 * K, HW)),
                "w2": w2,
            }
        )
    return in_maps


def run(x, offset, weight, trace=False, **kw):
    from concourse import bass_utils

    if "nc" not in _CACHE:
        _CACHE["nc"] = build()
    nc = _CACHE["nc"]
    res = bass_utils.run_bass_kernel_spmd(
        nc, prep_in_maps(x, offset, weight), core_ids=list(range(NCORES)),
        trace=trace, **kw,
    )
    out = np.stack([r["out"].reshape(O, H, W) for r in res.results])
    return out, res


def kernel(x, offset, weight):
    out, _ = run(x, offset, weight, trace=False)
    return out



# revision 5
# speedup vs baseline: 1.2089x; 1.2089x over previous
"""DeformConv (B=8, C=256, H=W=64, O=256, 3x3, DG=1) Trainium2 Bass kernel.

Sharding: data-parallel over batch, one batch element per NeuronCore (8 cores).

Per-core pipeline (B=1):
  1. x [256,4096] f32 -> fp16 (SWDGE cast-load, padded cols) -> 4 shifted-
     window PE transpose passes (col offsets 0/1/64/65) build 2x2-patch rows
     [pos, 4*256ch] directly in SBUF; two big contiguous HWDGE writes put the
     table in DRAM in permuted row order r = (lin%128)*32 + lin//128 (64KB
     per partition -> ~130 descriptors instead of ~22K 512B lines).
  2. Coords on DVE (f32): y0=floor(sy) (magic-number round + is_gt fix),
     base row r=clip(y0,0,62), col b=clip(x0,0,62), separable slot weights
     wsy[2], wsx[2] reproducing mmcv zero-padding bilinear exactly; the
     gather index is the permuted row id r.
  3. Gather: one dma_gather per (1024-pos chunk, tap): 2KB elems from
     x_patch, spread over 4 SWDGE queues (lane-consistent, load-balanced).
     Output [pos%128, pos//128, 4*256].
  4. Blend: per corner, broadcast ws along channels (free-step-0 AP copy)
     then big [128,8,256] tensor_tensor mult/add on DVE.
  5. PE-transpose blended [pos,ch]->[ch,pos] (fp16) + ACT copies PSUM->SBUF.
  6. GEMM out[o,p] = sum_{c,k} W[o,c,k]*sampled[c,k,p]: 18 contraction
     blocks of 128, fp16 operands, f32 PSUM, interleaved into the tap loop.
"""

import dataclasses

import numpy as np

_CACHE = {}

H = 64
W = 64
HW = 4096
C = 256
O = 256
K = 9
NCORES = 8
NQ = 4  # SWDGE queues
MAGIC = float(3 << 22)  # 1.5*2^23: keeps x+MAGIC in [2^23, 2^24) for |x|<2^22
USE_ILGEMM = True  # interleave GEMM accumulation into the tap loop


def _step0(ap, inner):
    """Expand a [128, n] AP to [128, n, inner] with stride-0 inner dim."""
    return dataclasses.replace(ap, ap=list(ap.ap) + [[0, inner]])


def _emit(tc, nc, aps, rec=None, queue_plan=None):
    import contextlib

    import concourse.bass as bass
    import concourse.mybir as mybir
    from concourse.masks import make_identity

    dt = mybir.dt
    Alu = mybir.AluOpType
    Act = mybir.ActivationFunctionType

    x_in = aps["x"]          # [256, 4096] f32
    off_in = aps["offset"]   # [18, 4096]  f32
    w2_in = aps["w2"]        # [2304, 256] f32   (k-major, then c; lhsT layout)
    out_d = aps["out"]       # [256, 4096] f32

    ctx = contextlib.ExitStack()
    with ctx:
        # ---------------- pools ----------------
        cpool = ctx.enter_context(tc.tile_pool(name="cpool", bufs=1))
        dpool = ctx.enter_context(tc.tile_pool(name="dpool", bufs=1, space="DRAM"))

        # ---------------- persistent tiles ----------------
        ident16 = cpool.tile([128, 128], dt.float16, name="ident16")
        ident32 = cpool.tile([128, 128], dt.float32, name="ident32")
        make_identity(nc, ident16)
        make_identity(nc, ident32)

        w2_sb = cpool.tile([128, 18, 256], dt.float16, name="w2_sb")
        _i = nc.gpsimd.dma_start(
            out=w2_sb, in_=w2_in.rearrange("(kb ci) o -> ci kb o", ci=128)
        )
        if rec is not None:
            rec["plain"].append(_i.ins if hasattr(_i, "ins") else _i)
        # slot-weight fields: corners 1-3 stored as adjacent PAIRS so the
        # blend mults' in1 AP has innermost step-1 (enables DVE 2x_1P mode);
        # corner 0 stays f32 per-partition scalars for the ACT path.
        ws16p = cpool.tile([128, 3, K * 32, 2], dt.float16, name="ws16p")
        ws32 = cpool.tile([128, 1, K * 32], dt.float32, name="ws32")
        idxw = cpool.tile([128, K * 4 * 64], dt.int16, name="idxw")

        x_patch = dpool.tile([HW, 1024], dt.float16, name="x_patch")

        eng = [nc.sync, nc.scalar]

        # ================= PREP PHASE (scoped pools) =================
        with tc.tile_pool(name="prep", bufs=1) as pp, tc.tile_pool(
            name="ppsum", bufs=2, space="PSUM"
        ) as pps:
            # ---- offsets -> p-major layout via PE transpose ----
            off_sb = pp.tile([18, HW], dt.float32, name="off_sb")
            nc.sync.dma_start(out=off_sb, in_=off_in)
            offp = pp.tile([128, 32, 18], dt.float32, name="offp")
            for i in range(32):
                pso = pps.tile([128, 18], dt.float32, name="pso", tag="pso")
                nc.tensor.transpose(
                    pso, off_sb[:, i * 128 : (i + 1) * 128], ident32[0:18, 0:18]
                )
                nc.vector.tensor_copy(offp[:, i, :], pso)

            # ---- position iota ----
            pos_i = pp.tile([128, 32], dt.int32, name="pos_i")
            nc.gpsimd.iota(pos_i, pattern=[[128, 32]], base=0, channel_multiplier=1)
            POS = pp.tile([128, 32], dt.float32, name="POS")
            nc.vector.tensor_copy(POS, pos_i)
            Pq = pp.tile([128, 32], dt.float32, name="Pq")
            nc.vector.tensor_scalar(Pq, POS, 1.0 / 64.0, None, Alu.mult)
            I_ = pp.tile([128, 32], dt.float32, name="I_")
            CMP = pp.tile([128, 32], dt.float32, name="CMPij")
            nc.vector.tensor_scalar(CMP, Pq, MAGIC, None, Alu.add)
            nc.vector.tensor_scalar(I_, CMP, MAGIC, None, Alu.subtract)
            nc.vector.tensor_tensor(CMP, I_, Pq, Alu.is_gt)
            nc.vector.tensor_tensor(I_, I_, CMP, Alu.subtract)
            J_ = pp.tile([128, 32], dt.float32, name="J_")
            nc.vector.scalar_tensor_tensor(J_, I_, -64.0, POS, Alu.mult, Alu.add)

            # ---- x cast-load (fp16, padded cols so shifted windows stay
            # in-bounds), then 4 shifted-window transpose passes build the
            # 2x2-patch rows [pos, (slot,ch)] straight in SBUF ----
            XP = HW + 128
            x16 = pp.tile([128, 2, XP], dt.float16, name="x16")
            nc.gpsimd.memset(x16[:, :, HW:XP], 0)
            _i = nc.gpsimd.dma_start(
                out=x16[:, :, 0:HW], in_=x_in.rearrange("(h c) p -> c h p", h=2)
            )
            if rec is not None:
                rec["plain"].append(_i.ins if hasattr(_i, "ins") else _i)
            patch_sb = pp.tile([128, 32, 1024], dt.float16, name="patch_sb")
            for i in range(32):
                xtp = pps.tile([128, 1024], dt.float16, name="xtp", tag="xtp")
                for s, sh in enumerate((0, 1, 64, 65)):
                    for h in range(2):
                        nc.tensor.transpose(
                            xtp[:, s * 256 + h * 128 : s * 256 + (h + 1) * 128],
                            x16[:, h, i * 128 + sh : i * 128 + sh + 128],
                            ident16,
                        )
                if i % 2 == 0:
                    nc.scalar.activation(patch_sb[:, i, :], xtp, Act.Copy)
                else:
                    nc.vector.tensor_copy(patch_sb[:, i, :], xtp)
            # big contiguous writes: partition j, block i -> row j*32+i.
            # 32KB runs per descriptor (HW caps payload strictly below 64KB).
            for half in range(2):
                p0 = half * 64
                dst = bass.AP(
                    tensor=x_patch.tensor,
                    offset=x_patch.offset + p0 * 32 * 1024,
                    ap=[[32 * 1024, 64], [16 * 1024, 2], [1, 16 * 1024]],
                )
                src = dataclasses.replace(
                    patch_sb[p0 : p0 + 64, :, :],
                    ap=[
                        [patch_sb.ap[0][0], 64],
                        [16 * 1024, 2],
                        [1, 16 * 1024],
                    ],
                )
                eng[half].dma_start(out=dst, in_=src)

            # ---- per-axis coordinate pipeline ----
            KI = [k // 3 for k in range(K)]
            KJ = [k % 3 for k in range(K)]

            def axis_pipeline(off_field, base_tile, kshift, L, WS0, WS1, R_out):
                F = K * 32
                S = pp.tile([128, F], dt.float32, name=f"S{L}", tag=f"S{L}")
                for k in range(K):
                    nc.vector.scalar_tensor_tensor(
                        S[:, k * 32 : (k + 1) * 32],
                        off_field(k),
                        float(kshift[k] - 1),
                        base_tile,
                        Alu.add,
                        Alu.add,
                    )
                t = lambda nm: pp.tile([128, F], dt.float32, name=nm, tag=nm)
                Y0 = t(f"Y0{L}")
                Ct = t(f"Ct{L}")
                nc.vector.tensor_scalar(Ct, S, MAGIC, None, Alu.add)
                nc.vector.tensor_scalar(Y0, Ct, MAGIC, None, Alu.subtract)
                nc.vector.tensor_tensor(Ct, Y0, S, Alu.is_gt)
                nc.vector.tensor_tensor(Y0, Y0, Ct, Alu.subtract)
                LY = t(f"LY{L}")
                nc.vector.tensor_tensor(LY, S, Y0, Alu.subtract)
                WY0 = t(f"WY0{L}")
                nc.vector.tensor_scalar(WY0, LY, -1.0, 1.0, Alu.mult, Alu.add)
                V0 = t(f"V0{L}")
                V1 = t(f"V1{L}")
                nc.vector.tensor_scalar(V0, Y0, 0.0, None, Alu.is_ge)
                nc.vector.tensor_scalar(Ct, Y0, 63.0, None, Alu.is_le)
                nc.vector.tensor_tensor(V0, V0, Ct, Alu.mult)
                nc.vector.tensor_scalar(V1, Y0, -1.0, None, Alu.is_ge)
                nc.vector.tensor_scalar(Ct, Y0, 62.0, None, Alu.is_le)
                nc.vector.tensor_tensor(V1, V1, Ct, Alu.mult)
                nc.vector.tensor_tensor(WY0, WY0, V0, Alu.mult)
                nc.vector.tensor_tensor(LY, LY, V1, Alu.mult)
                R = R_out
                nc.vector.tensor_scalar(R, Y0, 0.0, 62.0, Alu.max, Alu.min)
                C0 = t(f"C0{L}")
                C1 = t(f"C1{L}")
                nc.vector.tensor_scalar(C0, Y0, 0.0, 63.0, Alu.max, Alu.min)
                nc.vector.tensor_scalar(C1, Y0, 1.0, 0.0, Alu.add, Alu.max)
                nc.vector.tensor_scalar(C1, C1, 63.0, None, Alu.min)
                E = t(f"E{L}")
                T1 = t(f"T1{L}")
                nc.vector.tensor_tensor(E, C0, R, Alu.is_equal)
                nc.vector.tensor_tensor(T1, WY0, E, Alu.mult)
                nc.vector.tensor_tensor(E, C1, R, Alu.is_equal)
                nc.vector.tensor_tensor(E, LY, E, Alu.mult)
                nc.vector.tensor_tensor(WS0, T1, E, Alu.add)
                Rp = t(f"Rp{L}")
                nc.vector.tensor_scalar(Rp, R, 1.0, None, Alu.add)
                nc.vector.tensor_tensor(E, C0, Rp, Alu.is_equal)
                nc.vector.tensor_tensor(T1, WY0, E, Alu.mult)
                nc.vector.tensor_tensor(E, C1, Rp, Alu.is_equal)
                nc.vector.tensor_tensor(E, LY, E, Alu.mult)
                nc.vector.tensor_tensor(WS1, T1, E, Alu.add)

            F = K * 32
            WSY0 = pp.tile([128, F], dt.float32, name="WSY0")
            WSY1 = pp.tile([128, F], dt.float32, name="WSY1")
            WSX0 = pp.tile([128, F], dt.float32, name="WSX0")
            WSX1 = pp.tile([128, F], dt.float32, name="WSX1")
            RY = pp.tile([128, F], dt.float32, name="RY")
            RX = pp.tile([128, F], dt.float32, name="RX")
            axis_pipeline(lambda k: offp[:, :, 2 * k], I_, KI, "y", WSY0, WSY1, RY)
            axis_pipeline(
                lambda k: offp[:, :, 2 * k + 1], J_, KJ, "x", WSX0, WSX1, RX
            )
            WSf = pp.tile([128, F], dt.float32, name="WSf", tag="WSf")
            nc.vector.tensor_tensor(ws32[:, 0, :], WSY0, WSX0, Alu.mult)
            for st, (wy, wx) in enumerate(
                [(None, None), (WSY0, WSX1), (WSY1, WSX0), (WSY1, WSX1)]
            ):
                if st == 0:
                    continue
                nc.vector.tensor_tensor(WSf, wy, wx, Alu.mult)
                nc.vector.tensor_copy(ws16p[:, st - 1, :, 0], WSf)
                nc.vector.tensor_copy(ws16p[:, st - 1, :, 1], WSf)

            # ---- gather indices: lin = RY*64+RX, permuted row id
            # r = (lin%128)*32 + lin//128, cast to i16 ----
            IDX = pp.tile([128, 384], dt.float32, name="IDX")
            nc.gpsimd.memset(IDX, 0)
            LIN = pp.tile([128, F], dt.float32, name="LIN")
            nc.vector.scalar_tensor_tensor(
                LIN, RY, 64.0, RX, Alu.mult, Alu.add
            )
            PQ = pp.tile([128, F], dt.float32, name="PQ")
            QQ = pp.tile([128, F], dt.float32, name="QQ")
            CT = pp.tile([128, F], dt.float32, name="CT2")
            nc.vector.tensor_scalar(PQ, LIN, 1.0 / 128.0, None, Alu.mult)
            nc.vector.tensor_scalar(CT, PQ, MAGIC, None, Alu.add)
            nc.vector.tensor_scalar(QQ, CT, MAGIC, None, Alu.subtract)
            nc.vector.tensor_tensor(CT, QQ, PQ, Alu.is_gt)
            nc.vector.tensor_tensor(QQ, QQ, CT, Alu.subtract)
            nc.vector.scalar_tensor_tensor(CT, QQ, -128.0, LIN, Alu.mult, Alu.add)
            nc.vector.scalar_tensor_tensor(
                IDX[:, 0:F], CT, 32.0, QQ, Alu.mult, Alu.add
            )
            # shuffle p%128 -> p%16 wrap via two PE transpose stages (f32),
            # casting to i16 on the final PSUM->SBUF copy:
            # idxw[t, (k,ch)*64 + bl*8 + g] = IDX[g*16+t, k*32+ch*8+bl]
            t1sb = pp.tile([128, 3, 128], dt.float32, name="t1sb")
            for ct in range(3):
                ps1 = pps.tile([128, 128], dt.float32, name="ps1", tag="ps1")
                nc.tensor.transpose(ps1, IDX[:, ct * 128 : (ct + 1) * 128], ident32)
                nc.vector.tensor_copy(t1sb[:, ct, :], ps1)
            # stage 2: per (ct, g): [128col, 16] -> [16, 128col]
            for ct in range(3):
                nk = 4 if ct < 2 else 1  # k-count covered by this col tile
                for g in range(8):
                    ps2 = pps.tile([16, 128], dt.float32, name="ps2", tag="ps2")
                    nc.tensor.transpose(
                        ps2, t1sb[:, ct, g * 16 : (g + 1) * 16], ident32
                    )
                    # dst cols: for k' in [0,nk), ch in 4, bl in 8:
                    #   ((ct*4+k')*4+ch)*64 + bl*8 + g
                    dst = bass.AP(
                        tensor=idxw.tensor,
                        offset=idxw.offset + (ct * 4 * 4) * 64 + g,
                        ap=[[idxw.ap[0][0], 16], [256, nk], [64, 4], [8, 8]],
                    )
                    nc.vector.tensor_copy(
                        dst,
                        ps2[0:16, 0 : nk * 32].rearrange(
                            "t (k c b) -> t k c b", k=nk, c=4
                        ),
                    )
            # replicate the wrapped idx table to all 8 partition groups on
            # the sync/scalar HWDGE queues, keeping the SWDGE lanes free of
            # plain dma_starts (lets gathers spread across queues).
            for rep in range(1, 8):
                eng[rep % 2].dma_start(
                    out=idxw[rep * 16 : (rep + 1) * 16, :], in_=idxw[0:16, :]
                )

        # ================= MAIN LOOP =================
        pspool = ctx.enter_context(tc.tile_pool(name="pspool", bufs=3, space="PSUM"))
        psg = ctx.enter_context(tc.tile_pool(name="psg", bufs=1, space="PSUM"))
        gpool = ctx.enter_context(tc.tile_pool(name="gpool", bufs=3))
        spool = ctx.enter_context(tc.tile_pool(name="spool", bufs=2))
        bpool = ctx.enter_context(tc.tile_pool(name="bpool", bufs=3))
        opool = ctx.enter_context(tc.tile_pool(name="opool", bufs=3))

        for ch in range(4):  # 1024-position chunks
            S = [
                spool.tile([128, 1024], dt.float16, name=f"S{kb}", tag=f"S{kb}")
                for kb in range(18)
            ]
            pgs = [
                psg.tile([128, 512], dt.float32, name=f"pg{j}", tag=f"pg{j}")
                for j in range(4)
            ]
            for k in range(K):
                G = gpool.tile([128, 8, 1024], dt.float16, name="G", tag="G", bufs=3)
                qi = ch * K + k
                qn = 0 if queue_plan is None else queue_plan[qi]
                _i = nc.gpsimd.dma_gather(
                    G,
                    x_patch,
                    idxw[:, (k * 4 + ch) * 64 : (k * 4 + ch + 1) * 64],
                    num_idxs=1024,
                    num_idxs_reg=1024,
                    elem_size=1024,
                    elem_step=1024,
                    queue_num=qn,
                )
                if rec is not None:
                    rec["gather"].append(_i.ins if hasattr(_i, "ins") else _i)
                # blend 4 corners: A = sum_st ws_st * G[:, :, st].
                # corner 0 products on ACT (per-partition scale, per-bl ops);
                # corners 1-3 on DVE as fused broadcast-mults (step-0 in1).
                A = bpool.tile([128, 8, 256], dt.float16, name="A", tag="A")
                Mt = bpool.tile([128, 8, 256], dt.float16, name="Mt", tag="Mt")
                P0 = bpool.tile([128, 8, 256], dt.float16, name="P0", tag="P0")
                for bl in range(8):
                    wc = k * 32 + ch * 8 + bl
                    nc.scalar.activation(
                        P0[:, bl, :],
                        G[:, bl, 0:256],
                        Act.Copy,
                        scale=ws32[:, 0, wc : wc + 1],
                    )
                for st in range(1, 4):
                    # in1: [part, bl(x2 step), 128(x step0), pair(step1)] —
                    # innermost step-1 fp16 pairs keep DVE in 2x_1P mode.
                    wsl = ws16p[:, st - 1, k * 32 + ch * 8 : k * 32 + ch * 8 + 8, :]
                    wpair = dataclasses.replace(
                        wsl, ap=[wsl.ap[0], [2, 8], [0, 128], [1, 2]]
                    )
                    dst = Mt if st > 1 else A
                    nc.vector.tensor_tensor(
                        dst,
                        G[:, :, st * 256 : (st + 1) * 256],
                        wpair,
                        Alu.mult,
                    )
                    if st == 1:
                        nc.vector.tensor_tensor(A, A, P0, Alu.add)
                    else:
                        nc.vector.tensor_tensor(A, A, Mt, Alu.add)
                # transpose [pos, ch] -> [ch, pos]
                for h in range(2):
                    for blq in range(2):
                        pt = pspool.tile(
                            [128, 512], dt.float16, name="pt", tag="pt", bufs=3
                        )
                        for bb in range(4):
                            bl = blq * 4 + bb
                            nc.tensor.transpose(
                                pt[:, bb * 128 : (bb + 1) * 128],
                                A[:, bl, h * 128 : (h + 1) * 128],
                                ident16,
                            )
                        nc.scalar.activation(
                            S[k * 2 + h][:, blq * 512 : (blq + 1) * 512],
                            pt,
                            Act.Copy,
                        )
                # interleaved GEMM, 1-tap delayed: tap k's 2 kb-blocks are
                # contracted while tap k+1's transposes provide PE slack,
                # so matmuls never wait on the same tap's PSUM->SBUF copies.
                # 8 [128x512] matmuls per tap keeps PE load even.
                if USE_ILGEMM:
                    gk = [k - 1] if k >= 1 else []
                    if k == K - 1:
                        gk.append(k)
                    for kk in gk:
                        for m in range(2):
                            for sub in range(2):
                                for kb in (kk * 2, kk * 2 + 1):
                                    nc.tensor.matmul(
                                        pgs[m * 2 + sub],
                                        lhsT=w2_sb[:, kb, m * 128 : (m + 1) * 128],
                                        rhs=S[kb][:, sub * 512 : (sub + 1) * 512],
                                        start=(kb == 0),
                                        stop=(kb == 17),
                                    )
            if not USE_ILGEMM:
                for m in range(2):
                    for sub in range(2):
                        for kb in range(18):
                            nc.tensor.matmul(
                                pgs[m * 2 + sub],
                                lhsT=w2_sb[:, kb, m * 128 : (m + 1) * 128],
                                rhs=S[kb][:, sub * 512 : (sub + 1) * 512],
                                start=(kb == 0),
                                stop=(kb == 17),
                            )
            for m in range(2):
                for sub in range(2):
                    ot = opool.tile([128, 512], dt.float32, name="ot", tag="ot")
                    nc.vector.tensor_copy(ot, pgs[m * 2 + sub])
                    eng[(m * 2 + sub) % 2].dma_start(
                        out=out_d[
                            m * 128 : (m + 1) * 128,
                            ch * 1024 + sub * 512 : ch * 1024 + (sub + 1) * 512,
                        ],
                        in_=ot,
                    )


def _lane_of(inst):
    from concourse.tile_sem_assignment import PROC_NAME_TO_IDX

    rev = {v: k for k, v in PROC_NAME_TO_IDX.items()}
    nm = rev.get(inst.bass_scheduled_proc, "")
    return int(nm[5:]) if nm.startswith("DMASW") else None


def build(queue_plan="auto"):
    import concourse.mybir as mybir
    from concourse import bacc, tile

    dt = mybir.dt
    nc = bacc.Bacc(
        "TRN2",
        target_bir_lowering=False,
        debug=False,
        enable_asserts=False,
        num_devices=NCORES,
        num_swdge_queues=NQ,
    )
    aps = {
        "x": nc.dram_tensor("x", [C, HW], dt.float32, kind="ExternalInput").ap(),
        "offset": nc.dram_tensor(
            "offset", [2 * K, HW], dt.float32, kind="ExternalInput"
        ).ap(),
        "w2": nc.dram_tensor(
            "w2", [C * K, O], dt.float32, kind="ExternalInput"
        ).ap(),
        "out": nc.dram_tensor(
            "out", [O, HW], dt.float32, kind="ExternalOutput"
        ).ap(),
    }
    if queue_plan == "auto":
        # pass 1: discover each SWDGE DMA's DMASW lane, then rebuild with a
        # lane-consistent, load-balanced queue assignment (lanes that host
        # plain queue-0 dma_starts are forced to 0; the rest are packed onto
        # queues 0..NQ-1 greedily by gather count).
        rec = {"gather": [], "plain": []}
        with tile.TileContext(nc) as tc:
            _emit(tc, nc, aps, rec=rec, queue_plan=None)
        plain_lanes = {_lane_of(i) for i in rec["plain"]} - {None}
        lanes = [_lane_of(i) for i in rec["gather"]]
        counts = {}
        for ln in lanes:
            if ln is not None and ln not in plain_lanes:
                counts[ln] = counts.get(ln, 0) + 1
        load = [0] * NQ
        load[0] += sum(1 for ln in lanes if ln is None or ln in plain_lanes)
        lane_q = {}
        for ln in sorted(counts, key=lambda l: -counts[l]):
            q = min(range(NQ), key=lambda j: load[j])
            lane_q[ln] = q
            load[q] += counts[ln]
        plan = []
        for ln in lanes:
            if ln is None or ln in plain_lanes:
                plan.append(0)
            else:
                plan.append(lane_q[ln])
        return build(plan)
    with tile.TileContext(nc) as tc:
        _emit(tc, nc, aps, queue_plan=queue_plan)
    nc.compile()
    return nc


def prep_in_maps(x, offset, weight):
    x = np.asarray(x, dtype=np.float32)
    offset = np.asarray(offset, dtype=np.float32)
    weight = np.asarray(weight, dtype=np.float32)
    w2 = np.ascontiguousarray(
        weight.reshape(O, C, K).transpose(2, 1, 0).reshape(C * K, O)
    )
    in_maps = []
    for b in range(NCORES):
        in_maps.append(
            {
                "x": np.ascontiguousarray(x[b].reshape(C, HW)),
                "offset": np.ascontiguousarray(offset[b].reshape(2 * K, HW)),
                "w2": w2,
            }
        )
    return in_maps


def run(x, offset, weight, trace=False, **kw):
    from concourse import bass_utils

    if "nc" not in _CACHE:
        _CACHE["nc"] = build()
    nc = _CACHE["nc"]
    res = bass_utils.run_bass_kernel_spmd(
        nc, prep_in_maps(x, offset, weight), core_ids=list(range(NCORES)),
        trace=trace, **kw,
    )
    out = np.stack([r["out"].reshape(O, H, W) for r in res.results])
    return out, res


def kernel(x, offset, weight):
    out, _ = run(x, offset, weight, trace=False)
    return out


# revision 12
# speedup vs baseline: 1.3455x; 1.1130x over previous
"""DeformConv (B=8, C=256, H=W=64, O=256, 3x3, DG=1) Trainium2 Bass kernel.

Sharding: data-parallel over batch, one batch element per NeuronCore (8 cores).

Per-core pipeline (B=1):
  1. x [256,4096] f32 -> fp16 (SWDGE cast-load, padded cols) -> 4 shifted-
     window PE transpose passes (col offsets 0/1/64/65) build 2x2-patch rows
     [pos, 4*256ch] directly in SBUF; two big contiguous HWDGE writes put the
     table in DRAM in permuted row order r = (lin%128)*32 + lin//128 (64KB
     per partition -> ~130 descriptors instead of ~22K 512B lines).
  2. Coords on DVE (f32): y0=floor(sy) (magic-number round + is_gt fix),
     base row r=clip(y0,0,62), col b=clip(x0,0,62), separable slot weights
     wsy[2], wsx[2] reproducing mmcv zero-padding bilinear exactly; the
     gather index is the permuted row id r.
  3. Gather: one dma_gather per (1024-pos chunk, tap): 2KB elems from
     x_patch, spread over 4 SWDGE queues (lane-consistent, load-balanced).
     Output [pos%128, pos//128, 4*256].
  4. Blend: per corner, broadcast ws along channels (free-step-0 AP copy)
     then big [128,8,256] tensor_tensor mult/add on DVE.
  5. PE-transpose blended [pos,ch]->[ch,pos] (fp16) + ACT copies PSUM->SBUF.
  6. GEMM out[o,p] = sum_{c,k} W[o,c,k]*sampled[c,k,p]: 18 contraction
     blocks of 128, fp16 operands, f32 PSUM, interleaved into the tap loop.
"""

import dataclasses

import numpy as np

_CACHE = {}

H = 64
W = 64
HW = 4096
C = 256
O = 256
K = 9
NCORES = 8
NQ = 4  # SWDGE queues
MAGIC = float(3 << 22)  # 1.5*2^23: keeps x+MAGIC in [2^23, 2^24) for |x|<2^22
USE_ILGEMM = True  # interleave GEMM accumulation into the tap loop


def _step0(ap, inner):
    """Expand a [128, n] AP to [128, n, inner] with stride-0 inner dim."""
    return dataclasses.replace(ap, ap=list(ap.ap) + [[0, inner]])


def _emit(tc, nc, aps, rec=None, queue_plan=None):
    import contextlib

    import concourse.bass as bass
    import concourse.mybir as mybir
    from concourse.masks import make_identity

    dt = mybir.dt
    Alu = mybir.AluOpType
    Act = mybir.ActivationFunctionType

    x_in = aps["x"]          # [256, 4096] f32
    off_in = aps["offset"]   # [18, 4096]  f32
    w2_in = aps["w2"]        # [2304, 256] f32   (k-major, then c; lhsT layout)
    out_d = aps["out"]       # [256, 4096] f32

    ctx = contextlib.ExitStack()
    with ctx:
        # ---------------- pools ----------------
        cpool = ctx.enter_context(tc.tile_pool(name="cpool", bufs=1))
        dpool = ctx.enter_context(tc.tile_pool(name="dpool", bufs=1, space="DRAM"))

        # ---------------- persistent tiles ----------------
        ident16 = cpool.tile([128, 128], dt.float16, name="ident16")
        ident32 = cpool.tile([128, 128], dt.float32, name="ident32")
        make_identity(nc, ident16)
        make_identity(nc, ident32)

        w2_sb = cpool.tile([128, 18, 256], dt.float16, name="w2_sb")
        _i = nc.gpsimd.dma_start(
            out=w2_sb, in_=w2_in.rearrange("(kb ci) o -> ci kb o", ci=128)
        )
        if rec is not None:
            rec["plain"].append(_i.ins if hasattr(_i, "ins") else _i)
        # slot-weight fields: corners 1-3 stored as adjacent PAIRS so the
        # blend mults' in1 AP has innermost step-1 (enables DVE 2x_1P mode);
        # corner 0 stays f32 per-partition scalars for the ACT path.
        ws16p = cpool.tile([128, 3, K * 32, 2], dt.float16, name="ws16p")
        ws32 = cpool.tile([128, 1, K * 32], dt.float32, name="ws32")
        idxw = cpool.tile([128, K * 4 * 64], dt.int16, name="idxw")

        x_patch = dpool.tile([HW, 1024], dt.float16, name="x_patch")

        eng = [nc.sync, nc.scalar]

        # ================= PREP PHASE (scoped pools) =================
        with tc.tile_pool(name="prep", bufs=1) as pp, tc.tile_pool(
            name="ppsum", bufs=2, space="PSUM"
        ) as pps:
            # ---- offsets -> p-major layout via PE transpose ----
            off_sb = pp.tile([18, HW], dt.float32, name="off_sb")
            nc.sync.dma_start(out=off_sb, in_=off_in)
            offp = pp.tile([128, 32, 18], dt.float32, name="offp")
            for i in range(32):
                pso = pps.tile([128, 18], dt.float32, name="pso", tag="pso")
                nc.tensor.transpose(
                    pso, off_sb[:, i * 128 : (i + 1) * 128], ident32[0:18, 0:18]
                )
                nc.vector.tensor_copy(offp[:, i, :], pso)

            # ---- position iota ----
            pos_i = pp.tile([128, 32], dt.int32, name="pos_i")
            nc.gpsimd.iota(pos_i, pattern=[[128, 32]], base=0, channel_multiplier=1)
            POS = pp.tile([128, 32], dt.float32, name="POS")
            nc.vector.tensor_copy(POS, pos_i)
            Pq = pp.tile([128, 32], dt.float32, name="Pq")
            nc.vector.tensor_scalar(Pq, POS, 1.0 / 64.0, None, Alu.mult)
            I_ = pp.tile([128, 32], dt.float32, name="I_")
            CMP = pp.tile([128, 32], dt.float32, name="CMPij")
            nc.vector.tensor_scalar(CMP, Pq, MAGIC, None, Alu.add)
            nc.vector.tensor_scalar(I_, CMP, MAGIC, None, Alu.subtract)
            nc.vector.tensor_tensor(CMP, I_, Pq, Alu.is_gt)
            nc.vector.tensor_tensor(I_, I_, CMP, Alu.subtract)
            J_ = pp.tile([128, 32], dt.float32, name="J_")
            nc.vector.scalar_tensor_tensor(J_, I_, -64.0, POS, Alu.mult, Alu.add)

            # ---- x cast-load (fp16, padded cols so shifted windows stay
            # in-bounds), then 4 shifted-window transpose passes build the
            # 2x2-patch rows [pos, (slot,ch)] straight in SBUF ----
            XP = HW + 128
            x16 = pp.tile([128, 2, XP], dt.float16, name="x16")
            nc.gpsimd.memset(x16[:, :, HW:XP], 0)
            _i = nc.gpsimd.dma_start(
                out=x16[:, :, 0:HW], in_=x_in.rearrange("(h c) p -> c h p", h=2)
            )
            if rec is not None:
                rec["plain"].append(_i.ins if hasattr(_i, "ins") else _i)
            patch_sb = pp.tile([128, 32, 1024], dt.float16, name="patch_sb")
            for i in range(32):
                xtp = pps.tile([128, 1024], dt.float16, name="xtp", tag="xtp")
                for s, sh in enumerate((0, 1, 64, 65)):
                    for h in range(2):
                        nc.tensor.transpose(
                            xtp[:, s * 256 + h * 128 : s * 256 + (h + 1) * 128],
                            x16[:, h, i * 128 + sh : i * 128 + sh + 128],
                            ident16,
                        )
                if i % 4 != 3:
                    nc.scalar.activation(patch_sb[:, i, :], xtp, Act.Copy)
                else:
                    nc.vector.tensor_copy(patch_sb[:, i, :], xtp)
            # big contiguous writes: partition j, block i -> row j*32+i.
            # 32KB runs per descriptor (HW caps payload strictly below 64KB).
            for half in range(2):
                p0 = half * 64
                dst = bass.AP(
                    tensor=x_patch.tensor,
                    offset=x_patch.offset + p0 * 32 * 1024,
                    ap=[[32 * 1024, 64], [16 * 1024, 2], [1, 16 * 1024]],
                )
                src = dataclasses.replace(
                    patch_sb[p0 : p0 + 64, :, :],
                    ap=[
                        [patch_sb.ap[0][0], 64],
                        [16 * 1024, 2],
                        [1, 16 * 1024],
                    ],
                )
                eng[half].dma_start(out=dst, in_=src)

            # ---- per-axis coordinate pipeline ----
            KI = [k // 3 for k in range(K)]
            KJ = [k % 3 for k in range(K)]

            def axis_pipeline(
                off_field, base_tile, kshift, L, WS0, WS1, R_out, ev=None
            ):
                ev = ev or nc.vector
                F = K * 32
                S = pp.tile([128, F], dt.float32, name=f"S{L}", tag=f"S{L}")
                for k in range(K):
                    ev.scalar_tensor_tensor(
                        S[:, k * 32 : (k + 1) * 32],
                        off_field(k),
                        float(kshift[k] - 1),
                        base_tile,
                        Alu.add,
                        Alu.add,
                    )
                t = lambda nm: pp.tile([128, F], dt.float32, name=nm, tag=nm)
                Y0 = t(f"Y0{L}")
                Ct = t(f"Ct{L}")
                ev.tensor_scalar(Ct, S, MAGIC, None, Alu.add)
                ev.tensor_scalar(Y0, Ct, MAGIC, None, Alu.subtract)
                ev.tensor_tensor(Ct, Y0, S, Alu.is_gt)
                ev.tensor_tensor(Y0, Y0, Ct, Alu.subtract)
                LY = t(f"LY{L}")
                ev.tensor_tensor(LY, S, Y0, Alu.subtract)
                WY0 = t(f"WY0{L}")
                ev.tensor_scalar(WY0, LY, -1.0, 1.0, Alu.mult, Alu.add)
                V0 = t(f"V0{L}")
                V1 = t(f"V1{L}")
                ev.tensor_scalar(V0, Y0, 0.0, None, Alu.is_ge)
                ev.tensor_scalar(Ct, Y0, 63.0, None, Alu.is_le)
                ev.tensor_tensor(V0, V0, Ct, Alu.mult)
                ev.tensor_scalar(V1, Y0, -1.0, None, Alu.is_ge)
                ev.tensor_scalar(Ct, Y0, 62.0, None, Alu.is_le)
                ev.tensor_tensor(V1, V1, Ct, Alu.mult)
                ev.tensor_tensor(WY0, WY0, V0, Alu.mult)
                ev.tensor_tensor(LY, LY, V1, Alu.mult)
                R = R_out
                ev.tensor_scalar(R, Y0, 0.0, 62.0, Alu.max, Alu.min)
                C0 = t(f"C0{L}")
                C1 = t(f"C1{L}")
                ev.tensor_scalar(C0, Y0, 0.0, 63.0, Alu.max, Alu.min)
                ev.tensor_scalar(C1, Y0, 1.0, 0.0, Alu.add, Alu.max)
                ev.tensor_scalar(C1, C1, 63.0, None, Alu.min)
                E = t(f"E{L}")
                T1 = t(f"T1{L}")
                ev.tensor_tensor(E, C0, R, Alu.is_equal)
                ev.tensor_tensor(T1, WY0, E, Alu.mult)
                ev.tensor_tensor(E, C1, R, Alu.is_equal)
                ev.tensor_tensor(E, LY, E, Alu.mult)
                ev.tensor_tensor(WS0, T1, E, Alu.add)
                Rp = t(f"Rp{L}")
                ev.tensor_scalar(Rp, R, 1.0, None, Alu.add)
                ev.tensor_tensor(E, C0, Rp, Alu.is_equal)
                ev.tensor_tensor(T1, WY0, E, Alu.mult)
                ev.tensor_tensor(E, C1, Rp, Alu.is_equal)
                ev.tensor_tensor(E, LY, E, Alu.mult)
                ev.tensor_tensor(WS1, T1, E, Alu.add)

            F = K * 32
            WSY0 = pp.tile([128, F], dt.float32, name="WSY0")
            WSY1 = pp.tile([128, F], dt.float32, name="WSY1")
            WSX0 = pp.tile([128, F], dt.float32, name="WSX0")
            WSX1 = pp.tile([128, F], dt.float32, name="WSX1")
            RY = pp.tile([128, F], dt.float32, name="RY")
            RX = pp.tile([128, F], dt.float32, name="RX")
            axis_pipeline(lambda k: offp[:, :, 2 * k], I_, KI, "y", WSY0, WSY1, RY)
            axis_pipeline(
                lambda k: offp[:, :, 2 * k + 1], J_, KJ, "x", WSX0, WSX1, RX
            )
            WSf = pp.tile([128, F], dt.float32, name="WSf", tag="WSf")
            nc.vector.tensor_tensor(ws32[:, 0, :], WSY0, WSX0, Alu.mult)
            for st, (wy, wx) in enumerate(
                [(None, None), (WSY0, WSX1), (WSY1, WSX0), (WSY1, WSX1)]
            ):
                if st == 0:
                    continue
                nc.vector.tensor_tensor(WSf, wy, wx, Alu.mult)
                nc.vector.tensor_copy(ws16p[:, st - 1, :, 0], WSf)
                nc.vector.tensor_copy(ws16p[:, st - 1, :, 1], WSf)

            # ---- gather indices: lin = RY*64+RX, permuted row id
            # r = (lin%128)*32 + lin//128, cast to i16 ----
            IDX = pp.tile([128, 384], dt.float32, name="IDX")
            nc.gpsimd.memset(IDX, 0)
            LIN = pp.tile([128, F], dt.float32, name="LIN")
            nc.vector.scalar_tensor_tensor(
                LIN, RY, 64.0, RX, Alu.mult, Alu.add
            )
            PQ = pp.tile([128, F], dt.float32, name="PQ")
            QQ = pp.tile([128, F], dt.float32, name="QQ")
            CT = pp.tile([128, F], dt.float32, name="CT2")
            nc.vector.tensor_scalar(PQ, LIN, 1.0 / 128.0, None, Alu.mult)
            nc.vector.tensor_scalar(CT, PQ, MAGIC, None, Alu.add)
            nc.vector.tensor_scalar(QQ, CT, MAGIC, None, Alu.subtract)
            nc.vector.tensor_tensor(CT, QQ, PQ, Alu.is_gt)
            nc.vector.tensor_tensor(QQ, QQ, CT, Alu.subtract)
            nc.vector.scalar_tensor_tensor(CT, QQ, -128.0, LIN, Alu.mult, Alu.add)
            nc.vector.scalar_tensor_tensor(
                IDX[:, 0:F], CT, 32.0, QQ, Alu.mult, Alu.add
            )
            # shuffle p%128 -> p%16 wrap via two PE transpose stages (f32),
            # casting to i16 on the final PSUM->SBUF copy:
            # idxw[t, (k,ch)*64 + bl*8 + g] = IDX[g*16+t, k*32+ch*8+bl]
            t1sb = pp.tile([128, 3, 128], dt.float32, name="t1sb")
            for ct in range(3):
                ps1 = pps.tile([128, 128], dt.float32, name="ps1", tag="ps1")
                nc.tensor.transpose(ps1, IDX[:, ct * 128 : (ct + 1) * 128], ident32)
                nc.vector.tensor_copy(t1sb[:, ct, :], ps1)
            # stage 2: per (ct, g): [128col, 16] -> [16, 128col]
            for ct in range(3):
                nk = 4 if ct < 2 else 1  # k-count covered by this col tile
                for g in range(8):
                    ps2 = pps.tile([16, 128], dt.float32, name="ps2", tag="ps2")
                    nc.tensor.transpose(
                        ps2, t1sb[:, ct, g * 16 : (g + 1) * 16], ident32
                    )
                    # dst cols: for k' in [0,nk), ch in 4, bl in 8:
                    #   ((ct*4+k')*4+ch)*64 + bl*8 + g
                    dst = bass.AP(
                        tensor=idxw.tensor,
                        offset=idxw.offset + (ct * 4 * 4) * 64 + g,
                        ap=[[idxw.ap[0][0], 16], [256, nk], [64, 4], [8, 8]],
                    )
                    nc.vector.tensor_copy(
                        dst,
                        ps2[0:16, 0 : nk * 32].rearrange(
                            "t (k c b) -> t k c b", k=nk, c=4
                        ),
                    )
            # replicate the wrapped idx table to all 8 partition groups on
            # the sync/scalar HWDGE queues, keeping the SWDGE lanes free of
            # plain dma_starts (lets gathers spread across queues).
            for rep in range(1, 8):
                eng[rep % 2].dma_start(
                    out=idxw[rep * 16 : (rep + 1) * 16, :], in_=idxw[0:16, :]
                )

        # ================= MAIN LOOP =================
        pspool = ctx.enter_context(tc.tile_pool(name="pspool", bufs=3, space="PSUM"))
        psg = ctx.enter_context(tc.tile_pool(name="psg", bufs=1, space="PSUM"))
        gpool = ctx.enter_context(tc.tile_pool(name="gpool", bufs=4))
        spool = ctx.enter_context(tc.tile_pool(name="spool", bufs=2))
        bpool = ctx.enter_context(tc.tile_pool(name="bpool", bufs=3))
        opool = ctx.enter_context(tc.tile_pool(name="opool", bufs=3))

        for ch in range(4):  # 1024-position chunks
            S = [
                spool.tile([128, 1024], dt.float16, name=f"S{kb}", tag=f"S{kb}")
                for kb in range(18)
            ]
            pgs = [
                psg.tile([128, 512], dt.float32, name=f"pg{j}", tag=f"pg{j}")
                for j in range(4)
            ]
            for k in range(K):
                G = gpool.tile([128, 8, 1024], dt.float16, name="G", tag="G", bufs=4)
                qi = ch * K + k
                qn = 0 if queue_plan is None else queue_plan[qi]
                _i = nc.gpsimd.dma_gather(
                    G,
                    x_patch,
                    idxw[:, (k * 4 + ch) * 64 : (k * 4 + ch + 1) * 64],
                    num_idxs=1024,
                    num_idxs_reg=1024,
                    elem_size=1024,
                    elem_step=1024,
                    queue_num=qn,
                )
                if rec is not None:
                    rec["gather"].append(_i.ins if hasattr(_i, "ins") else _i)
                # blend 4 corners: A = sum_st ws_st * G[:, :, st].
                # corner 0 products on ACT (per-partition scale, per-bl ops);
                # corners 1-3 on DVE as fused broadcast-mults (step-0 in1).
                A = bpool.tile([128, 8, 256], dt.float16, name="A", tag="A")
                Mt = bpool.tile([128, 8, 256], dt.float16, name="Mt", tag="Mt")
                P0 = bpool.tile([128, 8, 256], dt.float16, name="P0", tag="P0")
                for bl in range(8):
                    wc = k * 32 + ch * 8 + bl
                    nc.scalar.activation(
                        P0[:, bl, :],
                        G[:, bl, 0:256],
                        Act.Copy,
                        scale=ws32[:, 0, wc : wc + 1],
                    )
                for st in range(1, 4):
                    # in1: [part, bl(x2 step), 128(x step0), pair(step1)] —
                    # innermost step-1 fp16 pairs keep DVE in 2x_1P mode.
                    wsl = ws16p[:, st - 1, k * 32 + ch * 8 : k * 32 + ch * 8 + 8, :]
                    wpair = dataclasses.replace(
                        wsl, ap=[wsl.ap[0], [2, 8], [0, 128], [1, 2]]
                    )
                    dst = Mt if st > 1 else A
                    nc.vector.tensor_tensor(
                        dst,
                        G[:, :, st * 256 : (st + 1) * 256],
                        wpair,
                        Alu.mult,
                    )
                    if st == 1:
                        nc.vector.tensor_tensor(A, A, P0, Alu.add)
                    else:
                        nc.vector.tensor_tensor(A, A, Mt, Alu.add)
                # transpose [pos, ch] -> [ch, pos]
                for h in range(2):
                    for blq in range(2):
                        pt = pspool.tile(
                            [128, 512], dt.float16, name="pt", tag="pt", bufs=3
                        )
                        for bb in range(4):
                            bl = blq * 4 + bb
                            nc.tensor.transpose(
                                pt[:, bb * 128 : (bb + 1) * 128],
                                A[:, bl, h * 128 : (h + 1) * 128],
                                ident16,
                            )
                        nc.scalar.activation(
                            S[k * 2 + h][:, blq * 512 : (blq + 1) * 512],
                            pt,
                            Act.Copy,
                        )
                # interleaved GEMM, 1-tap delayed: tap k's 2 kb-blocks are
                # contracted while tap k+1's transposes provide PE slack,
                # so matmuls never wait on the same tap's PSUM->SBUF copies.
                # 8 [128x512] matmuls per tap keeps PE load even.
                if USE_ILGEMM:
                    gk = [k - 1] if k >= 1 else []
                    if k == K - 1:
                        gk.append(k)
                    for kk in gk:
                        for m in range(2):
                            for sub in range(2):
                                for kb in (kk * 2, kk * 2 + 1):
                                    nc.tensor.matmul(
                                        pgs[m * 2 + sub],
                                        lhsT=w2_sb[:, kb, m * 128 : (m + 1) * 128],
                                        rhs=S[kb][:, sub * 512 : (sub + 1) * 512],
                                        start=(kb == 0),
                                        stop=(kb == 17),
                                    )
            if not USE_ILGEMM:
                for m in range(2):
                    for sub in range(2):
                        for kb in range(18):
                            nc.tensor.matmul(
                                pgs[m * 2 + sub],
                                lhsT=w2_sb[:, kb, m * 128 : (m + 1) * 128],
                                rhs=S[kb][:, sub * 512 : (sub + 1) * 512],
                                start=(kb == 0),
                                stop=(kb == 17),
                            )
            for m in range(2):
                for sub in range(2):
                    ot = opool.tile([128, 512], dt.float32, name="ot", tag="ot")
                    nc.vector.tensor_copy(ot, pgs[m * 2 + sub])
                    eng[(m * 2 + sub) % 2].dma_start(
                        out=out_d[
                            m * 128 : (m + 1) * 128,
                            ch * 1024 + sub * 512 : ch * 1024 + (sub + 1) * 512,
                        ],
                        in_=ot,
                    )


def _lane_of(inst):
    from concourse.tile_sem_assignment import PROC_NAME_TO_IDX

    rev = {v: k for k, v in PROC_NAME_TO_IDX.items()}
    nm = rev.get(inst.bass_scheduled_proc, "")
    return int(nm[5:]) if nm.startswith("DMASW") else None


def build(queue_plan="auto"):
    import concourse.mybir as mybir
    from concourse import bacc, tile

    dt = mybir.dt
    nc = bacc.Bacc(
        "TRN2",
        target_bir_lowering=False,
        debug=False,
        enable_asserts=False,
        num_devices=NCORES,
        num_swdge_queues=NQ,
    )
    aps = {
        "x": nc.dram_tensor("x", [C, HW], dt.float32, kind="ExternalInput").ap(),
        "offset": nc.dram_tensor(
            "offset", [2 * K, HW], dt.float32, kind="ExternalInput"
        ).ap(),
        "w2": nc.dram_tensor(
            "w2", [C * K, O], dt.float32, kind="ExternalInput"
        ).ap(),
        "out": nc.dram_tensor(
            "out", [O, HW], dt.float32, kind="ExternalOutput"
        ).ap(),
    }
    if queue_plan == "auto":
        # pass 1: discover each SWDGE DMA's DMASW lane, then rebuild with a
        # lane-consistent, load-balanced queue assignment (lanes that host
        # plain queue-0 dma_starts are forced to 0; the rest are packed onto
        # queues 0..NQ-1 greedily by gather count).
        rec = {"gather": [], "plain": []}
        with tile.TileContext(nc) as tc:
            _emit(tc, nc, aps, rec=rec, queue_plan=None)
        plain_lanes = {_lane_of(i) for i in rec["plain"]} - {None}
        lanes = [_lane_of(i) for i in rec["gather"]]
        # lane % NQ keeps consecutive gathers (which round-robin the 8 DMASW
        # lanes) on cyclically different queues; lanes hosting plain queue-0
        # dma_starts are forced to 0.
        plan = [
            0 if (ln is None or ln in plain_lanes) else ln % NQ for ln in lanes
        ]
        return build(plan)
    with tile.TileContext(nc) as tc:
        _emit(tc, nc, aps, queue_plan=queue_plan)
    nc.compile()
    return nc


def prep_in_maps(x, offset, weight):
    x = np.asarray(x, dtype=np.float32)
    offset = np.asarray(offset, dtype=np.float32)
    weight = np.asarray(weight, dtype=np.float32)
    w2 = np.ascontiguousarray(
        weight.reshape(O, C, K).transpose(2, 1, 0).reshape(C * K, O)
    )
    in_maps = []
    for b in range(NCORES):
        in_maps.append(
            {
                "x": np.ascontiguousarray(x[b].reshape(C, HW)),
                "offset": np.ascontiguousarray(offset[b].reshape(2 * K, HW)),
                "w2": w2,
            }
        )
    return in_maps


def run(x, offset, weight, trace=False, **kw):
    from concourse import bass_utils

    if "nc" not in _CACHE:
        _CACHE["nc"] = build()
    nc = _CACHE["nc"]
    res = bass_utils.run_bass_kernel_spmd(
        nc, prep_in_maps(x, offset, weight), core_ids=list(range(NCORES)),
        trace=trace, **kw,
    )
    out = np.stack([r["out"].reshape(O, H, W) for r in res.results])
    return out, res


def kernel(x, offset, weight):
    out, _ = run(x, offset, weight, trace=False)
    return out


# revision 13
# speedup vs baseline: 1.4045x; 1.0438x over previous
"""DeformConv (B=8, C=256, H=W=64, O=256, 3x3, DG=1) Trainium2 Bass kernel.

Sharding: data-parallel over batch, one batch element per NeuronCore (8 cores).

Per-core pipeline (B=1):
  1. x [256,4096] f32 -> fp16 (SWDGE cast-load, padded cols) -> 4 shifted-
     window PE transpose passes (col offsets 0/1/64/65) build 2x2-patch rows
     [pos, 4*256ch] directly in SBUF; four contiguous HWDGE writes put the
     table in DRAM in permuted row order r = (lin%128)*32 + lin//128 (32KB
     descriptors instead of ~22K 512B lines).
  2. Coords on DVE (f32), split in two phases so the gather-index chain
     (phase A: S, floor, clipped row) runs ahead of the blend-weight math
     (phase B), which only has to beat the first blend, not the first gather.
  3. Gather: one dma_gather per (1024-pos chunk, tap): 2KB elems from
     x_patch, spread over 4 SWDGE queues (lane-consistent, cyclic).
     Output [pos%128, pos//128, 4*256].
  4. Blend: per corner, broadcast ws along channels (free-step-0 AP copy)
     then big [128,8,256] tensor_tensor mult/add on DVE.
  5. PE-transpose blended [pos,ch]->[ch,pos] (fp16) + ACT copies PSUM->SBUF.
  6. GEMM out[o,p] = sum_{c,k} W[o,c,k]*sampled[c,k,p]: 18 contraction
     blocks of 128, fp16 operands, f32 PSUM, interleaved into the tap loop.

Engine queues are in-order, so emission order is placement: everything the
first gather depends on (idx chain, patch build) is emitted before the
weight math, and PSUM-drain copies go to ACT so DVE never stalls between
chunks.
"""

import dataclasses

import numpy as np

_CACHE = {}

H = 64
W = 64
HW = 4096
C = 256
O = 256
K = 9
NCORES = 8
NQ = 4  # SWDGE queues
MAGIC = float(3 << 22)  # 1.5*2^23: keeps x+MAGIC in [2^23, 2^24) for |x|<2^22
USE_ILGEMM = True  # interleave GEMM accumulation into the tap loop


def _step0(ap, inner):
    """Expand a [128, n] AP to [128, n, inner] with stride-0 inner dim."""
    return dataclasses.replace(ap, ap=list(ap.ap) + [[0, inner]])


def _emit(tc, nc, aps, rec=None, queue_plan=None):
    import contextlib

    import concourse.bass as bass
    import concourse.mybir as mybir
    from concourse.masks import make_identity

    dt = mybir.dt
    Alu = mybir.AluOpType
    Act = mybir.ActivationFunctionType

    x_in = aps["x"]          # [256, 4096] f32
    off_in = aps["offset"]   # [18, 4096]  f32
    w2_in = aps["w2"]        # [2304, 256] f32   (k-major, then c; lhsT layout)
    out_d = aps["out"]       # [256, 4096] f32

    ctx = contextlib.ExitStack()
    with ctx:
        # ---------------- pools ----------------
        cpool = ctx.enter_context(tc.tile_pool(name="cpool", bufs=1))
        dpool = ctx.enter_context(tc.tile_pool(name="dpool", bufs=1, space="DRAM"))

        # ---------------- persistent tiles ----------------
        ident16 = cpool.tile([128, 128], dt.float16, name="ident16")
        ident32 = cpool.tile([128, 128], dt.float32, name="ident32")
        make_identity(nc, ident16)
        make_identity(nc, ident32)

        w2_sb = cpool.tile([128, 18, 256], dt.float16, name="w2_sb")
        # slot-weight fields: corners 1-3 stored as adjacent PAIRS so the
        # blend mults' in1 AP has innermost step-1 (enables DVE 2x_1P mode);
        # corner 0 stays f32 per-partition scalars for the ACT path.
        ws16p = cpool.tile([128, 3, K * 32, 2], dt.float16, name="ws16p")
        ws32 = cpool.tile([128, 1, K * 32], dt.float32, name="ws32")
        idxw = cpool.tile([128, K * 4 * 64], dt.int16, name="idxw")

        x_patch = dpool.tile([HW, 1024], dt.float16, name="x_patch")

        eng = [nc.sync, nc.scalar]

        # ================= PREP PHASE (scoped pools) =================
        with tc.tile_pool(name="prep", bufs=1) as pp, tc.tile_pool(
            name="ppsum", bufs=2, space="PSUM"
        ) as pps:
            F = K * 32
            KI = [k // 3 for k in range(K)]
            KJ = [k % 3 for k in range(K)]

            # ---- x cast-load first: keeps the SWDGE queue busy from t=0
            # and unblocks the patch transposes as early as possible ----
            XP = HW + 128
            x16 = pp.tile([128, 2, XP], dt.float16, name="x16")
            nc.gpsimd.memset(x16[:, :, HW:XP], 0)
            for colh in range(2):
                _i = nc.gpsimd.dma_start(
                    out=x16[:, :, colh * 2048 : (colh + 1) * 2048],
                    in_=x_in[:, colh * 2048 : (colh + 1) * 2048].rearrange(
                        "(h c) p -> c h p", h=2
                    ),
                )
                if rec is not None:
                    rec["plain"].append(_i.ins if hasattr(_i, "ins") else _i)
            IDX = pp.tile([128, 384], dt.float32, name="IDX")
            nc.gpsimd.memset(IDX, 0)

            # ---- offsets -> p-major layout via PE transpose ----
            off_sb = pp.tile([18, HW], dt.float32, name="off_sb")
            nc.sync.dma_start(out=off_sb, in_=off_in)
            offp = pp.tile([128, 32, 18], dt.float32, name="offp")
            for i in range(32):
                pso = pps.tile([128, 18], dt.float32, name="pso", tag="pso")
                nc.tensor.transpose(
                    pso, off_sb[:, i * 128 : (i + 1) * 128], ident32[0:18, 0:18]
                )
                nc.vector.tensor_copy(offp[:, i, :], pso)

            # ---- position iota ----
            pos_i = pp.tile([128, 32], dt.int32, name="pos_i")
            nc.gpsimd.iota(pos_i, pattern=[[128, 32]], base=0, channel_multiplier=1)
            POS = pp.tile([128, 32], dt.float32, name="POS")
            nc.vector.tensor_copy(POS, pos_i)
            Pq = pp.tile([128, 32], dt.float32, name="Pq")
            nc.vector.tensor_scalar(Pq, POS, 1.0 / 64.0, None, Alu.mult)
            I_ = pp.tile([128, 32], dt.float32, name="I_")
            CMP = pp.tile([128, 32], dt.float32, name="CMPij")
            nc.vector.tensor_scalar(CMP, Pq, MAGIC, None, Alu.add)
            nc.vector.tensor_scalar(I_, CMP, MAGIC, None, Alu.subtract)
            nc.vector.tensor_tensor(CMP, I_, Pq, Alu.is_gt)
            nc.vector.tensor_tensor(I_, I_, CMP, Alu.subtract)
            J_ = pp.tile([128, 32], dt.float32, name="J_")
            nc.vector.scalar_tensor_tensor(J_, I_, -64.0, POS, Alu.mult, Alu.add)

            # ---- coordinate phase A: sample coords, floor, clipped row ----
            AX = {}

            def axis_a(off_field, base_tile, kshift, L, R_out):
                S = pp.tile([128, F], dt.float32, name=f"S{L}")
                Y0 = pp.tile([128, F], dt.float32, name=f"Y0{L}")
                Ct = pp.tile([128, F], dt.float32, name=f"Ct{L}")
                for k in range(K):
                    nc.vector.scalar_tensor_tensor(
                        S[:, k * 32 : (k + 1) * 32],
                        off_field(k),
                        float(kshift[k] - 1),
                        base_tile,
                        Alu.add,
                        Alu.add,
                    )
                nc.vector.tensor_scalar(Ct, S, MAGIC, None, Alu.add)
                nc.vector.tensor_scalar(Y0, Ct, MAGIC, None, Alu.subtract)
                nc.vector.tensor_tensor(Ct, Y0, S, Alu.is_gt)
                nc.vector.tensor_tensor(Y0, Y0, Ct, Alu.subtract)
                nc.vector.tensor_scalar(R_out, Y0, 0.0, 62.0, Alu.max, Alu.min)
                AX[L] = (S, Y0, Ct)

            RY = pp.tile([128, F], dt.float32, name="RY")
            RX = pp.tile([128, F], dt.float32, name="RX")
            axis_a(lambda k: offp[:, :, 2 * k], I_, KI, "y", RY)
            axis_a(lambda k: offp[:, :, 2 * k + 1], J_, KJ, "x", RX)

            # ---- gather indices: lin = RY*64+RX, permuted row id
            # r = (lin%128)*32 + lin//128, cast to i16 ----
            LIN = pp.tile([128, F], dt.float32, name="LIN")
            nc.vector.scalar_tensor_tensor(
                LIN, RY, 64.0, RX, Alu.mult, Alu.add
            )
            PQ = pp.tile([128, F], dt.float32, name="PQ")
            QQ = pp.tile([128, F], dt.float32, name="QQ")
            CT = pp.tile([128, F], dt.float32, name="CT2")
            nc.vector.tensor_scalar(PQ, LIN, 1.0 / 128.0, None, Alu.mult)
            nc.vector.tensor_scalar(CT, PQ, MAGIC, None, Alu.add)
            nc.vector.tensor_scalar(QQ, CT, MAGIC, None, Alu.subtract)
            nc.vector.tensor_tensor(CT, QQ, PQ, Alu.is_gt)
            nc.vector.tensor_tensor(QQ, QQ, CT, Alu.subtract)
            nc.vector.scalar_tensor_tensor(CT, QQ, -128.0, LIN, Alu.mult, Alu.add)
            nc.vector.scalar_tensor_tensor(
                IDX[:, 0:F], CT, 32.0, QQ, Alu.mult, Alu.add
            )
            # shuffle p%128 -> p%16 wrap via two PE transpose stages (f32),
            # casting to i16 on the final PSUM->SBUF copy:
            # idxw[t, (k,ch)*64 + bl*8 + g] = IDX[g*16+t, k*32+ch*8+bl]
            t1sb = pp.tile([128, 3, 128], dt.float32, name="t1sb")
            for ct in range(3):
                ps1 = pps.tile([128, 128], dt.float32, name="ps1", tag="ps1")
                nc.tensor.transpose(ps1, IDX[:, ct * 128 : (ct + 1) * 128], ident32)
                nc.vector.tensor_copy(t1sb[:, ct, :], ps1)
            # stage 2: per (ct, g): [128col, 16] -> [16, 128col]
            for ct in range(3):
                nk = 4 if ct < 2 else 1  # k-count covered by this col tile
                for g in range(8):
                    ps2 = pps.tile([16, 128], dt.float32, name="ps2", tag="ps2")
                    nc.tensor.transpose(
                        ps2, t1sb[:, ct, g * 16 : (g + 1) * 16], ident32
                    )
                    # dst cols: for k' in [0,nk), ch in 4, bl in 8:
                    #   ((ct*4+k')*4+ch)*64 + bl*8 + g
                    dst = bass.AP(
                        tensor=idxw.tensor,
                        offset=idxw.offset + (ct * 4 * 4) * 64 + g,
                        ap=[[idxw.ap[0][0], 16], [256, nk], [64, 4], [8, 8]],
                    )
                    nc.vector.tensor_copy(
                        dst,
                        ps2[0:16, 0 : nk * 32].rearrange(
                            "t (k c b) -> t k c b", k=nk, c=4
                        ),
                    )
            # replicate the wrapped idx table to all 8 partition groups on
            # the sync/scalar HWDGE queues, keeping the SWDGE lanes free of
            # plain dma_starts (lets gathers spread across queues).
            for rep in range(1, 8):
                eng[rep % 2].dma_start(
                    out=idxw[rep * 16 : (rep + 1) * 16, :], in_=idxw[0:16, :]
                )

            # ---- 4 shifted-window transpose passes build the 2x2-patch
            # rows [pos, (slot,ch)] in SBUF (PSUM->SBUF copies on ACT so the
            # DVE queue stays clear for the idx chain above) ----
            patch_sb = pp.tile([128, 32, 1024], dt.float16, name="patch_sb")

            def patch_write(half, ihalf):
                # rows r = j*32+i: one 32KB descriptor per partition-run
                # (HW caps descriptor payloads strictly below 64KB)
                p0 = half * 64
                dst = bass.AP(
                    tensor=x_patch.tensor,
                    offset=x_patch.offset
                    + p0 * 32 * 1024
                    + ihalf * 16 * 1024,
                    ap=[[32 * 1024, 64], [1, 16 * 1024]],
                )
                src = dataclasses.replace(
                    patch_sb[p0 : p0 + 64, ihalf * 16 : (ihalf + 1) * 16, :],
                    ap=[[patch_sb.ap[0][0], 64], [1, 16 * 1024]],
                )
                eng[half].dma_start(out=dst, in_=src)

            for i in range(32):
                xtp = pps.tile([128, 1024], dt.float16, name="xtp", tag="xtp")
                for s, sh in enumerate((0, 1, 64, 65)):
                    for h in range(2):
                        nc.tensor.transpose(
                            xtp[:, s * 256 + h * 128 : s * 256 + (h + 1) * 128],
                            x16[:, h, i * 128 + sh : i * 128 + sh + 128],
                            ident16,
                        )
                nc.scalar.activation(patch_sb[:, i, :], xtp, Act.Copy)
                if i == 15:
                    patch_write(0, 0)
                    patch_write(1, 0)
                elif i == 31:
                    patch_write(0, 1)
                    patch_write(1, 1)

            # ---- weights load (SWDGE cast; only matmuls need it) ----
            _i = nc.gpsimd.dma_start(
                out=w2_sb, in_=w2_in.rearrange("(kb ci) o -> ci kb o", ci=128)
            )
            if rec is not None:
                rec["plain"].append(_i.ins if hasattr(_i, "ins") else _i)

            # ---- coordinate phase B: bilinear slot weights ----
            def axis_b(L, WS0, WS1, R):
                S, Y0, Ct = AX[L]
                t = lambda nm: pp.tile([128, F], dt.float32, name=nm, tag=nm)
                LY = t(f"LY{L}")
                nc.vector.tensor_tensor(LY, S, Y0, Alu.subtract)
                WY0 = t(f"WY0{L}")
                nc.vector.tensor_scalar(WY0, LY, -1.0, 1.0, Alu.mult, Alu.add)
                V0 = t(f"V0{L}")
                V1 = t(f"V1{L}")
                nc.vector.tensor_scalar(V0, Y0, 0.0, None, Alu.is_ge)
                nc.vector.tensor_scalar(Ct, Y0, 63.0, None, Alu.is_le)
                nc.vector.tensor_tensor(V0, V0, Ct, Alu.mult)
                nc.vector.tensor_scalar(V1, Y0, -1.0, None, Alu.is_ge)
                nc.vector.tensor_scalar(Ct, Y0, 62.0, None, Alu.is_le)
                nc.vector.tensor_tensor(V1, V1, Ct, Alu.mult)
                nc.vector.tensor_tensor(WY0, WY0, V0, Alu.mult)
                nc.vector.tensor_tensor(LY, LY, V1, Alu.mult)
                C0 = t(f"C0{L}")
                C1 = t(f"C1{L}")
                nc.vector.tensor_scalar(C0, Y0, 0.0, 63.0, Alu.max, Alu.min)
                nc.vector.tensor_scalar(C1, Y0, 1.0, 0.0, Alu.add, Alu.max)
                nc.vector.tensor_scalar(C1, C1, 63.0, None, Alu.min)
                E = t(f"E{L}")
                T1 = t(f"T1{L}")
                nc.vector.tensor_tensor(E, C0, R, Alu.is_equal)
                nc.vector.tensor_tensor(T1, WY0, E, Alu.mult)
                nc.vector.tensor_tensor(E, C1, R, Alu.is_equal)
                nc.vector.tensor_tensor(E, LY, E, Alu.mult)
                nc.vector.tensor_tensor(WS0, T1, E, Alu.add)
                Rp = t(f"Rp{L}")
                nc.vector.tensor_scalar(Rp, R, 1.0, None, Alu.add)
                nc.vector.tensor_tensor(E, C0, Rp, Alu.is_equal)
                nc.vector.tensor_tensor(T1, WY0, E, Alu.mult)
                nc.vector.tensor_tensor(E, C1, Rp, Alu.is_equal)
                nc.vector.tensor_tensor(E, LY, E, Alu.mult)
                nc.vector.tensor_tensor(WS1, T1, E, Alu.add)

            WSY0 = pp.tile([128, F], dt.float32, name="WSY0")
            WSY1 = pp.tile([128, F], dt.float32, name="WSY1")
            WSX0 = pp.tile([128, F], dt.float32, name="WSX0")
            WSX1 = pp.tile([128, F], dt.float32, name="WSX1")
            axis_b("y", WSY0, WSY1, RY)
            axis_b("x", WSX0, WSX1, RX)
            WSf = pp.tile([128, F], dt.float32, name="WSf", tag="WSf")
            nc.vector.tensor_tensor(ws32[:, 0, :], WSY0, WSX0, Alu.mult)
            for st, (wy, wx) in enumerate(
                [(None, None), (WSY0, WSX1), (WSY1, WSX0), (WSY1, WSX1)]
            ):
                if st == 0:
                    continue
                nc.vector.tensor_tensor(WSf, wy, wx, Alu.mult)
                nc.vector.tensor_copy(ws16p[:, st - 1, :, 0], WSf)
                nc.vector.tensor_copy(ws16p[:, st - 1, :, 1], WSf)

        # ================= MAIN LOOP =================
        pspool = ctx.enter_context(tc.tile_pool(name="pspool", bufs=4, space="PSUM"))
        psg = ctx.enter_context(tc.tile_pool(name="psg", bufs=1, space="PSUM"))
        gpool = ctx.enter_context(tc.tile_pool(name="gpool", bufs=4))
        spool = ctx.enter_context(tc.tile_pool(name="spool", bufs=2))
        bpool = ctx.enter_context(tc.tile_pool(name="bpool", bufs=3))
        opool = ctx.enter_context(tc.tile_pool(name="opool", bufs=3))

        for ch in range(4):  # 1024-position chunks
            S = [
                spool.tile([128, 1024], dt.float16, name=f"S{kb}", tag=f"S{kb}")
                for kb in range(18)
            ]
            pgs = [
                psg.tile([128, 512], dt.float32, name=f"pg{j}", tag=f"pg{j}")
                for j in range(4)
            ]
            for k in range(K):
                G = gpool.tile([128, 8, 1024], dt.float16, name="G", tag="G", bufs=4)
                qi = ch * K + k
                qn = 0 if queue_plan is None else queue_plan[qi]
                _i = nc.gpsimd.dma_gather(
                    G,
                    x_patch,
                    idxw[:, (k * 4 + ch) * 64 : (k * 4 + ch + 1) * 64],
                    num_idxs=1024,
                    num_idxs_reg=1024,
                    elem_size=1024,
                    elem_step=1024,
                    queue_num=qn,
                )
                if rec is not None:
                    rec["gather"].append(_i.ins if hasattr(_i, "ins") else _i)
                # blend 4 corners: A = sum_st ws_st * G[:, :, st].
                # corner 0 products on ACT (per-partition scale, per-bl ops);
                # corners 1-3 on DVE as fused broadcast-mults (step-0 in1).
                A = bpool.tile([128, 8, 256], dt.float16, name="A", tag="A")
                Mt = bpool.tile([128, 8, 256], dt.float16, name="Mt", tag="Mt")
                P0 = bpool.tile([128, 8, 256], dt.float16, name="P0", tag="P0")
                for bl in range(8):
                    wc = k * 32 + ch * 8 + bl
                    nc.scalar.activation(
                        P0[:, bl, :],
                        G[:, bl, 0:256],
                        Act.Copy,
                        scale=ws32[:, 0, wc : wc + 1],
                    )
                for st in range(1, 4):
                    # in1: [part, bl(x2 step), 128(x step0), pair(step1)] —
                    # innermost step-1 fp16 pairs keep DVE in 2x_1P mode.
                    wsl = ws16p[:, st - 1, k * 32 + ch * 8 : k * 32 + ch * 8 + 8, :]
                    wpair = dataclasses.replace(
                        wsl, ap=[wsl.ap[0], [2, 8], [0, 128], [1, 2]]
                    )
                    dst = Mt if st > 1 else A
                    nc.vector.tensor_tensor(
                        dst,
                        G[:, :, st * 256 : (st + 1) * 256],
                        wpair,
                        Alu.mult,
                    )
                    if st == 1:
                        nc.vector.tensor_tensor(A, A, P0, Alu.add)
                    else:
                        nc.vector.tensor_tensor(A, A, Mt, Alu.add)
                # transpose [pos, ch] -> [ch, pos]
                for h in range(2):
                    for blq in range(2):
                        pt = pspool.tile(
                            [128, 512], dt.float16, name="pt", tag="pt", bufs=4
                        )
                        for bb in range(4):
                            bl = blq * 4 + bb
                            nc.tensor.transpose(
                                pt[:, bb * 128 : (bb + 1) * 128],
                                A[:, bl, h * 128 : (h + 1) * 128],
                                ident16,
                            )
                        nc.scalar.activation(
                            S[k * 2 + h][:, blq * 512 : (blq + 1) * 512],
                            pt,
                            Act.Copy,
                        )
                # interleaved GEMM, 1-tap delayed: tap k's 2 kb-blocks are
                # contracted while tap k+1's transposes provide PE slack,
                # so matmuls never wait on the same tap's PSUM->SBUF copies.
                # 8 [128x512] matmuls per tap keeps PE load even.
                if USE_ILGEMM:
                    gk = [k - 1] if k >= 1 else []
                    if k == K - 1:
                        gk.append(k)
                    for kk in gk:
                        for m in range(2):
                            for sub in range(2):
                                for kb in (kk * 2, kk * 2 + 1):
                                    nc.tensor.matmul(
                                        pgs[m * 2 + sub],
                                        lhsT=w2_sb[:, kb, m * 128 : (m + 1) * 128],
                                        rhs=S[kb][:, sub * 512 : (sub + 1) * 512],
                                        start=(kb == 0),
                                        stop=(kb == 17),
                                    )
            if not USE_ILGEMM:
                for m in range(2):
                    for sub in range(2):
                        for kb in range(18):
                            nc.tensor.matmul(
                                pgs[m * 2 + sub],
                                lhsT=w2_sb[:, kb, m * 128 : (m + 1) * 128],
                                rhs=S[kb][:, sub * 512 : (sub + 1) * 512],
                                start=(kb == 0),
                                stop=(kb == 17),
                            )
            for m in range(2):
                for sub in range(2):
                    ot = opool.tile([128, 512], dt.float32, name="ot", tag="ot")
                    nc.scalar.activation(ot, pgs[m * 2 + sub], Act.Copy)
                    eng[(m * 2 + sub) % 2].dma_start(
                        out=out_d[
                            m * 128 : (m + 1) * 128,
                            ch * 1024 + sub * 512 : ch * 1024 + (sub + 1) * 512,
                        ],
                        in_=ot,
                    )


def _lane_of(inst):
    from concourse.tile_sem_assignment import PROC_NAME_TO_IDX

    rev = {v: k for k, v in PROC_NAME_TO_IDX.items()}
    nm = rev.get(inst.bass_scheduled_proc, "")
    return int(nm[5:]) if nm.startswith("DMASW") else None


def build(queue_plan="auto"):
    import concourse.mybir as mybir
    from concourse import bacc, tile

    dt = mybir.dt
    nc = bacc.Bacc(
        "TRN2",
        target_bir_lowering=False,
        debug=False,
        enable_asserts=False,
        num_devices=NCORES,
        num_swdge_queues=NQ,
    )
    aps = {
        "x": nc.dram_tensor("x", [C, HW], dt.float32, kind="ExternalInput").ap(),
        "offset": nc.dram_tensor(
            "offset", [2 * K, HW], dt.float32, kind="ExternalInput"
        ).ap(),
        "w2": nc.dram_tensor(
            "w2", [C * K, O], dt.float32, kind="ExternalInput"
        ).ap(),
        "out": nc.dram_tensor(
            "out", [O, HW], dt.float32, kind="ExternalOutput"
        ).ap(),
    }
    if queue_plan == "auto":
        # pass 1: discover each SWDGE DMA's DMASW lane, then rebuild with a
        # lane-consistent queue assignment: lane % NQ keeps consecutive
        # gathers (which round-robin the 8 DMASW lanes) on cyclically
        # different queues; lanes hosting plain queue-0 dma_starts -> 0.
        rec = {"gather": [], "plain": []}
        with tile.TileContext(nc) as tc:
            _emit(tc, nc, aps, rec=rec, queue_plan=None)
        plain_lanes = {_lane_of(i) for i in rec["plain"]} - {None}
        lanes = [_lane_of(i) for i in rec["gather"]]
        plan = [
            0 if (ln is None or ln in plain_lanes) else ln % NQ for ln in lanes
        ]
        return build(plan)
    with tile.TileContext(nc) as tc:
        _emit(tc, nc, aps, queue_plan=queue_plan)
    nc.compile()
    return nc


def prep_in_maps(x, offset, weight):
    x = np.asarray(x, dtype=np.float32)
    offset = np.asarray(offset, dtype=np.float32)
    weight = np.asarray(weight, dtype=np.float32)
    w2 = np.ascontiguousarray(
        weight.reshape(O, C, K).transpose(2, 1, 0).reshape(C * K, O)
    )
    in_maps = []
    for b in range(NCORES):
        in_maps.append(
            {
                "x": np.ascontiguousarray(x[b].reshape(C, HW)),
                "offset": np.ascontiguousarray(offset[b].reshape(2 * K, HW)),
                "w2": w2,
            }
        )
    return in_maps


def run(x, offset, weight, trace=False, **kw):
    from concourse import bass_utils

    if "nc" not in _CACHE:
        _CACHE["nc"] = build()
    nc = _CACHE["nc"]
    res = bass_utils.run_bass_kernel_spmd(
        nc, prep_in_maps(x, offset, weight), core_ids=list(range(NCORES)),
        trace=trace, **kw,
    )
    out = np.stack([r["out"].reshape(O, H, W) for r in res.results])
    return out, res


def kernel(x, offset, weight):
    out, _ = run(x, offset, weight, trace=False)
    return out
